# revision 1
# baseline (speedup 1.0000x reference)
"""Trainium2 Bass kernel for a 2-layer "BiGRU" (batch-flipped, per reference).

Structure exploited:
  * The reference's "backward" direction flips the BATCH dim, not time. In
    flipped coordinates (track hb_hat[b] := hb[B-1-b]) every GRU cell
    consumes the UNFLIPPED input stream; flips appear only when building
    layer-1's input concat and in the final output (host side).
  * Batch 64 is sharded over 8 cores in flip-closed groups of 8, so the
    flip is a local batch reversal and cores are fully independent.
  * The input-side matmuls (x@wihT + biases) are bulk-precomputed at full
    PE utilization: layer-0's in a prepass over all steps; layer-1's in
    16-step chunks as the layer-0 wavefront produces its inputs. The
    recurrent loop streams only whh through the PE (h^T stationary), with
    all four GRU cells in different PE column groups (output partition
    bases 0/32/64/96) so their weight streams run concurrently. Layer 1
    runs LAG steps behind layer 0 in the same iteration, sharing one PSUM
    tile, so the elementwise gate math covers all four cells per op.

Self-contained: hardcodes all shapes from the problem spec.
"""

import numpy as np

from concourse import bacc, tile
from concourse.bass import mybir

SEQ, BATCH, IN, HID = 512, 64, 512, 512
G3 = 3 * HID  # 1536
BC = 8        # local batch per core
NCORES = 8
CH = 16       # wavefront chunk (steps) for layer-1 input bulk matmuls
LAG = 20      # layer-1 lag behind layer-0 (> CH + bulk spread, multiple of W4)
W4 = 2        # gi DMA window (steps)
FP32 = mybir.dt.float32
BF16 = mybir.dt.bfloat16

# cell name, PSUM base partition, input K-chunks of 128
CELLS = [("f0", 0, 4), ("b0", 32, 4), ("f1", 64, 8), ("b1", 96, 8)]


def _blob_layout():
    """Free-dim offsets (in bf16 elements) inside the single load blob."""
    off = {}
    cur = 0
    for cname, _, kx in CELLS:
        for k in range(kx):
            off[f"w_{cname}{k}"] = cur
            cur += G3
        for k in range(4):
            off[f"u_{cname}{k}"] = cur
            cur += G3
    off["bias"] = cur      # rows 0:4 = cells; per cell: [bulk_bias 1536 | bhn 512]
    cur += 2048
    off["ohrow"] = cur     # rows 0:4; cell c: cols c*128..+128 = 1.0 in row c
    cur += 4 * 128
    off["i8"] = cur        # identity 8x8 replicated at partition bands 0/32/64/96
    cur += 8
    off["i40"] = cur       # 40x40 identity at partition rows 0:40 and 64:104
    cur += 40
    off["bsel"] = cur      # (4, 2*40) bias-row selectors: layer l cols l*40..
    cur += 80
    return off, cur


def build_core_program(S, repeats=1):
    assert S % CH == 0 and LAG % W4 == 0 and CH % W4 == 0
    nc = bacc.Bacc(None, target_bir_lowering=False)

    off, totw = _blob_layout()
    blob_d = nc.declare_dram_parameter("blob", [128, totw // 2], FP32, isOutput=False)
    xTp_d = nc.declare_dram_parameter("xTp", [128, 4, S * BC // 2], FP32, isOutput=False)
    out_d = nc.declare_dram_parameter("out", [S, 40, HID], BF16, isOutput=True)

    with tile.TileContext(nc) as tc:
        for _ in range(repeats):
            build_body(nc, tc, S, blob_d, xTp_d, out_d, off, totw)
    nc.compile()
    return nc


def build_body(nc, tc, S, blob_d, xTp_d, out_d, off, totw):
    import contextlib

    ACT = mybir.ActivationFunctionType
    OP = mybir.AluOpType
    NCHUNK = S // CH

    ctx = contextlib.ExitStack()
    with ctx:
        const = ctx.enter_context(tc.tile_pool(name="const", bufs=1))
        ghp = ctx.enter_context(tc.tile_pool(name="ghp", bufs=1, space="PSUM"))
        scr = ctx.enter_context(tc.tile_pool(name="scr", bufs=1, space="PSUM"))
        dram = ctx.enter_context(tc.tile_pool(name="dram", bufs=1, space="DRAM"))
        xr_pool = ctx.enter_context(tc.tile_pool(name="xr", bufs=2))
        ev_pool = ctx.enter_context(tc.tile_pool(name="ev", bufs=2))
        ring_pool = ctx.enter_context(tc.tile_pool(name="ring", bufs=3))
        buf_pool = ctx.enter_context(tc.tile_pool(name="buf", bufs=3))
        hT1_pool = ctx.enter_context(tc.tile_pool(name="hT1", bufs=3))
        g_pool = ctx.enter_context(tc.tile_pool(name="g", bufs=2))

        # ---- load blob (single DMA), bf16 views via bitcast ----
        blob = const.tile([128, totw // 2], FP32, tag="blob")
        nc.gpsimd.dma_start(out=blob[:], in_=blob_d[:])
        b16 = blob[:].bitcast(BF16)

        W, U, BULKB, BHN, OHR, I8 = {}, {}, {}, {}, {}, {}
        ob, oh, oi = off["bias"], off["ohrow"], off["i8"]
        for ci, (cname, base, kx) in enumerate(CELLS):
            W[cname] = [b16[:, off[f"w_{cname}{k}"]:off[f"w_{cname}{k}"] + G3]
                        for k in range(kx)]
            U[cname] = [b16[:, off[f"u_{cname}{k}"]:off[f"u_{cname}{k}"] + G3]
                        for k in range(4)]
            BULKB[cname] = b16[0:4, ob:ob + 1536]      # row ci is live
            BHN[cname] = b16[0:4, ob + 1536:ob + 2048]
            OHR[cname] = b16[0:4, oh + ci * 128:oh + (ci + 1) * 128]
            I8[cname] = b16[base:base + 8, oi:oi + 8]

        zero16 = const.tile([128, 512], BF16, tag="zero16")
        nc.any.memset(zero16[:], 0.0)
        oi40 = off["i40"]
        IDT = {0: b16[0:40, oi40:oi40 + 40], 1: b16[64:104, oi40:oi40 + 40]}
        obs = off["bsel"]
        BSEL = {l: b16[0:4, obs + l * 40:obs + (l + 1) * 40] for l in range(2)}
        BHNROWS = b16[0:4, ob + 1536:ob + 2048]

        # per-cell bulk bias row selector uses rows 0:4 of the bias segment;
        # BULKB/BHN slices are shared APs — the one-hot picks the row.

        # ---- internal DRAM for bulk gi results ----
        gi0_dram = {c: dram.tile([S * BC, G3], BF16, tag=f"gi0_{c}", name=f"gi0_{c}", uniquify=True)
                    for c in ("f0", "b0")}
        gi1_dram = {c: [dram.tile([CH * BC, G3], BF16, tag=f"gi1_{c}", bufs=4,
                                  name=f"gi1_{c}_{cc}")
                        for cc in range(NCHUNK)]
                    for c in ("f1", "b1")}

        def bulk_gi(cell, lhs_chunks, out_rows_ap):
            """One 128-row tile of gi = x @ wihT + bias -> DRAM (bf16)."""
            kx = len(lhs_chunks)
            for n in range(3):
                ps = scr.tile([128, 512], FP32, tag="scr", bufs=2)
                for k in range(kx):
                    nc.tensor.matmul(out=ps[:], lhsT=lhs_chunks[k],
                                     rhs=W[cell][k][:, n * 512:(n + 1) * 512],
                                     start=(k == 0), stop=False)
                nc.tensor.matmul(out=ps[:], lhsT=OHR[cell],
                                 rhs=BULKB[cell][:, n * 512:(n + 1) * 512],
                                 start=False, stop=True)
                ev = ev_pool.tile([128, 512], BF16, tag="ev")
                nc.vector.tensor_copy(out=ev[:], in_=ps[:])
                nc.gpsimd.dma_start(out=out_rows_ap[:, n * 512:(n + 1) * 512],
                                    in_=ev[:])

        # ---- prepass: gi0 for all steps ----
        for rt in range(S * BC // 128):
            xrt = xr_pool.tile([128, 4, 64], FP32, tag="xr")
            nc.gpsimd.dma_start(out=xrt[:], in_=xTp_d[:, :, rt * 64:(rt + 1) * 64])
            x16 = xrt[:].bitcast(BF16)   # (128, 4, 128)
            for cell in ("f0", "b0"):
                bulk_gi(cell, [x16[:, k, :] for k in range(4)],
                        gi0_dram[cell][rt * 128:(rt + 1) * 128, :])

        # ---- wavefront loop: L0 at step i, L1 at step i-LAG.
        # The two layers are fully independent chains (separate PSUM tiles
        # and gate ops) so their serial recurrence paths overlap on the
        # engines instead of concatenating. ----
        bufA, bufB, bufC = {}, {}, {}   # chunk -> (128, 4, CH*BC) tiles
        hT1_tiles = {}                  # t1 -> (128, 4, 64) tile
        zpair = (zero16[:, 0:256], zero16[:, 256:512])
        h_prev = {0: zpair, 1: zpair}

        def dma_ring(iw):
            """Prefetch one W4-step window of gi slices for both layers."""
            t0w, t1w = iw, iw - LAG
            r = ring_pool.tile([128, W4 * G3], BF16, tag="ring",
                               name=f"ring{iw}")
            if iw < 3 * W4:
                # first touch of each of the two ring slots: zero everything
                # so the K=40 injects never contract over garbage rows
                nc.any.memset(r[:], 0.0)
            rv = r.rearrange("p (s g) -> p s g", s=W4)
            if 0 <= t0w < S:
                for cell, base in (("f0", 0), ("b0", 32)):
                    src = gi0_dram[cell][:].rearrange(
                        "(s b) g -> b s g", b=BC)[:, t0w:t0w + W4, :]
                    nc.sync.dma_start(out=rv[base:base + BC], in_=src)
            if 0 <= t1w < S:
                for cell, base in (("f1", 64), ("b1", 96)):
                    src = gi1_dram[cell][t1w // CH][:].rearrange(
                        "(s b) g -> b s g", b=BC)[:, t1w % CH:t1w % CH + W4, :]
                    nc.sync.dma_start(out=rv[base:base + BC], in_=src)
            return r

        ring_next = dma_ring(0)
        ring = None
        for i in range(S + LAG):
            t0, t1 = i, i - LAG         # layer-0 / layer-1 step indices
            cc0 = t0 // CH

            if i % W4 == 0:
                ring = ring_next
                ring_next = dma_ring(i + W4) if i + W4 < S + LAG else None
            wi = i % W4

            if t0 < S and t0 % CH == 0:
                bufA[cc0] = buf_pool.tile([128, 4, CH * BC], BF16, tag="bufA",
                                          name=f"bufA{cc0}")
                bufB[cc0] = buf_pool.tile([128, 4, CH * BC], BF16, tag="bufB",
                                          name=f"bufB{cc0}")
                bufC[cc0] = buf_pool.tile([128, 4, CH * BC], BF16, tag="bufC",
                                          name=f"bufC{cc0}")

            def hch(cell, k):
                if cell == "f0":
                    cb, sl = bufA[(t0 - 1) // CH], ((t0 - 1) % CH) * BC
                    return cb[:, k, sl:sl + BC]
                if cell == "b0":
                    cb, sl = bufB[(t0 - 1) // CH], ((t0 - 1) % CH) * BC
                    return cb[:, k, sl:sl + BC]
                hb = 0 if cell == "f1" else 32
                return hT1_tiles[t1 - 1][k // 2][:, k % 2, hb:hb + BC]

            ghs = {}
            for layer, tl, cells in ((0, t0, (("f0", 0), ("b0", 32))),
                                     (1, t1, (("f1", 64), ("b1", 96)))):
                if not (0 <= tl < S):
                    continue
                lo = 0 if layer == 0 else 64
                gh = ghp.tile([128, G3], FP32, tag=f"gh{layer}", bufs=1,
                              name=f"gh{layer}_{i}")
                ghs[layer] = gh

                # r/z slices: one K=40 identity inject covers both cells'
                # bands, then 4 whh chunks per cell.
                for n, col in ((0, 0), (1, 512)):
                    nc.tensor.matmul(
                        out=gh[lo:lo + 40, col:col + 512], lhsT=IDT[layer],
                        rhs=ring[lo:lo + 40, wi * G3 + col:wi * G3 + col + 512],
                        start=True, stop=(tl == 0),
                        tile_position=(lo, lo))
                    if tl > 0:
                        for k in range(4):
                            for cell, base in cells:
                                nc.tensor.matmul(
                                    out=gh[base:base + BC, col:col + 512],
                                    lhsT=hch(cell, k),
                                    rhs=U[cell][k][:, col:col + 512],
                                    start=False, stop=(k == 3),
                                    tile_position=(0, base))
                # hn slice: one selector-matrix bias inject + 4 whh chunks
                nc.tensor.matmul(
                    out=gh[lo:lo + 40, 1024:1536], lhsT=BSEL[layer],
                    rhs=BHNROWS, start=True, stop=(tl == 0),
                    tile_position=(0, lo))
                if tl > 0:
                    for k in range(4):
                        for cell, base in cells:
                            nc.tensor.matmul(
                                out=gh[base:base + BC, 1024:1536],
                                lhsT=hch(cell, k),
                                rhs=U[cell][k][:, 1024:1536],
                                start=False, stop=(k == 3),
                                tile_position=(0, base))

            for layer, tl, cells in ((0, t0, (("f0", 0), ("b0", 32))),
                                     (1, t1, (("f1", 64), ("b1", 96)))):
                if not (0 <= tl < S):
                    continue
                lo = 0 if layer == 0 else 64
                gh = ghs[layer]

                # ---------- gates in two hidden halves, each on its OWN
                # tiles so dependency tracking releases half-0's stationary
                # chunks (k=0,1) before half-1's gates finish ----------
                P = slice(lo, lo + 40)
                if layer == 0:
                    sl = (t0 % CH) * BC
                else:
                    hT1 = (hT1_pool.tile([128, 2, 64], BF16, tag="hT1a",
                                         name=f"hT1a_{i}"),
                           hT1_pool.tile([128, 2, 64], BF16, tag="hT1b",
                                         name=f"hT1b_{i}"))
                    hT1_tiles[t1] = hT1
                h2pair = []
                for hf in (0, 1):
                    rzh = g_pool.tile([128, 512], BF16, tag=f"rz{layer}{hf}")
                    u16 = g_pool.tile([128, 256], BF16, tag=f"gtA{layer}{hf}")
                    v16 = g_pool.tile([128, 256], BF16, tag=f"gtB{layer}{hf}")
                    n16 = g_pool.tile([128, 256], BF16, tag=f"n16{layer}{hf}")
                    d16 = g_pool.tile([128, 256], BF16, tag=f"gtA{layer}{hf}")
                    e16 = g_pool.tile([128, 256], BF16, tag=f"gtB{layer}{hf}")
                    h2h = g_pool.tile([128, 256], BF16, tag=f"h2{layer}{hf}")
                    ptrh = scr.tile([128, 2, 64], BF16, tag="scr", bufs=2,
                                    name=f"ptr{layer}{hf}_{i}")
                    nc.scalar.activation(rzh[P, 0:256],
                                         gh[P, hf * 256:hf * 256 + 256],
                                         ACT.Sigmoid)
                    nc.vector.tensor_tensor(
                        out=u16[P, :], in0=rzh[P, 0:256],
                        in1=gh[P, 1024 + hf * 256:1280 + hf * 256], op=OP.mult)
                    nc.scalar.activation(rzh[P, 256:512],
                                         gh[P, 512 + hf * 256:768 + hf * 256],
                                         ACT.Sigmoid)
                    nc.vector.tensor_tensor(
                        out=v16[P, :], in0=u16[P, :],
                        in1=ring[P, wi * G3 + 1024 + hf * 256:
                                 wi * G3 + 1280 + hf * 256], op=OP.add)
                    nc.scalar.activation(n16[P, :], v16[P, :], ACT.Tanh)
                    nc.vector.tensor_tensor(out=d16[P, :],
                                            in0=h_prev[layer][hf][P, :],
                                            in1=n16[P, :], op=OP.subtract)
                    nc.vector.tensor_tensor(out=e16[P, :], in0=rzh[P, 256:512],
                                            in1=d16[P, :], op=OP.mult)
                    nc.vector.tensor_tensor(out=h2h[P, :], in0=n16[P, :],
                                            in1=e16[P, :], op=OP.add)
                    h2pair.append(h2h)
                    for kk in (0, 1):
                        nc.tensor.transpose(out=ptrh[:, kk, 0:40],
                                            in_=h2h[P, kk * 128:(kk + 1) * 128],
                                            identity=IDT[layer])
                    KH = slice(2 * hf, 2 * hf + 2)
                    if layer == 0:
                        nc.vector.tensor_copy(out=bufA[cc0][:, KH, sl:sl + BC],
                                              in_=ptrh[:, :, 0:BC])
                        nc.vector.tensor_copy(out=bufB[cc0][:, KH, sl:sl + BC],
                                              in_=ptrh[:, :, 32:32 + BC])
                        nc.vector.tensor_copy(out=bufC[cc0][:, KH, sl:sl + BC],
                                              in_=ptrh[:, :, 39:31:-1])
                    else:
                        nc.vector.tensor_copy(out=hT1[hf][:, :, 0:40],
                                              in_=ptrh[:, :, 0:40])
                h_prev[layer] = h2pair
                if layer == 1:
                    nc.gpsimd.dma_start(out=out_d[t1, :, 0:256],
                                        in_=h2pair[0][64:104, :])
                    nc.gpsimd.dma_start(out=out_d[t1, :, 256:512],
                                        in_=h2pair[1][64:104, :])

            # ---------- bulk gi1, spread one n-slice group per iteration ----
            bc = (t0 - (CH - 1)) // CH          # chunk completed CH-1 iters ago
            ph = (t0 - (CH - 1)) % CH
            if 0 <= bc < NCHUNK and ph < 3:
                lhs = [bufA[bc][:, k, :] for k in range(4)] + \
                      [bufC[bc][:, k, :] for k in range(4)]
                for gidx in (ph * 2, ph * 2 + 1):
                    cell = ("f1", "b1")[gidx // 3]
                    n = gidx % 3
                    ps = scr.tile([128, 512], FP32, tag="scr", bufs=2)
                    for k in range(8):
                        nc.tensor.matmul(out=ps[:], lhsT=lhs[k],
                                         rhs=W[cell][k][:, n * 512:(n + 1) * 512],
                                         start=(k == 0), stop=False)
                    nc.tensor.matmul(out=ps[:], lhsT=OHR[cell],
                                     rhs=BULKB[cell][:, n * 512:(n + 1) * 512],
                                     start=False, stop=True)
                    ev = ev_pool.tile([128, 512], BF16, tag="ev")
                    nc.vector.tensor_copy(out=ev[:], in_=ps[:])
                    nc.gpsimd.dma_start(
                        out=gi1_dram[cell][bc][:, n * 512:(n + 1) * 512], in_=ev[:])


# ---------------------------------------------------------------------------
# host side
# ---------------------------------------------------------------------------

_CACHE = {}


def _groups():
    return [list(range(4 * d, 4 * d + 4)) + [63 - (4 * d + 3), 63 - (4 * d + 2),
            63 - (4 * d + 1), 63 - 4 * d] for d in range(NCORES)]


def _bf16_u16(a):
    a = np.ascontiguousarray(a, np.float32)
    u = a.view(np.uint32)
    return ((u + 0x7FFF + ((u >> 16) & 1)) >> 16).astype(np.uint16)


def _pack_words(u16):
    ev = u16[..., 0::2].astype(np.uint32)
    od = u16[..., 1::2].astype(np.uint32)
    return (ev | (od << 16)).view(np.float32)


def _blob_host(inputs):
    off, totw = _blob_layout()
    blob = np.zeros((128, totw), np.uint16)
    for ci, (cname, base, kx) in enumerate(CELLS):
        wih = np.asarray(inputs[f"wih_{cname}"], np.float32)   # (1536, in)
        whh = np.asarray(inputs[f"whh_{cname}"], np.float32)   # (1536, 512)
        bih = np.asarray(inputs[f"bih_{cname}"], np.float32)
        bhh = np.asarray(inputs[f"bhh_{cname}"], np.float32)
        wt = _bf16_u16(wih.T.reshape(kx, 128, G3))
        ut = _bf16_u16(whh.T.reshape(4, 128, G3))
        for k in range(kx):
            o = off[f"w_{cname}{k}"]
            blob[:, o:o + G3] = wt[k]
        for k in range(4):
            o = off[f"u_{cname}{k}"]
            blob[:, o:o + G3] = ut[k]
        bulkb = np.concatenate([(bih + bhh)[:1024], bih[1024:]])
        blob[ci, off["bias"]:off["bias"] + 1536] = _bf16_u16(bulkb)
        blob[ci, off["bias"] + 1536:off["bias"] + 2048] = _bf16_u16(bhh[1024:])
        blob[ci, off["ohrow"] + ci * 128:off["ohrow"] + (ci + 1) * 128] = \
            _bf16_u16(np.ones(128, np.float32))
    one = _bf16_u16(np.ones(1, np.float32))[0]
    for base in (0, 32, 64, 96):
        for j in range(8):
            blob[base + j, off["i8"] + j] = one
    for base in (0, 64):
        for j in range(40):
            blob[base + j, off["i40"] + j] = one
    for l, (cf, cb) in enumerate(((0, 1), (2, 3))):
        blob[cf, off["bsel"] + l * 40:off["bsel"] + l * 40 + 8] = one
        blob[cb, off["bsel"] + l * 40 + 32:off["bsel"] + l * 40 + 40] = one
    return _pack_words(blob)


def _in_maps(inputs):
    S = inputs["x"].shape[0]
    x = np.asarray(inputs["x"], np.float32)
    groups = _groups()
    blob = _blob_host(inputs)
    in_maps = []
    for d in range(NCORES):
        xl = x[:, groups[d], :]                       # (S, 8, 512)
        # xTp layout: (128 part, 4 k, S*BC) -> words
        xT = _bf16_u16(xl.transpose(2, 0, 1).reshape(4, 128, S * BC))
        xT = np.ascontiguousarray(xT.transpose(1, 0, 2))   # (128, 4, S*BC)
        in_maps.append({"blob": blob, "xTp": _pack_words(xT)})
    return in_maps


def _assemble(outs, S):
    groups = _groups()
    out = np.zeros((S, BATCH, 2 * HID), np.float32)
    for d in range(NCORES):
        raw = np.asarray(outs[d]["out"], np.float32)  # (S, 40, 512)
        G = groups[d]
        for b in range(BC):
            out[:, G[b], 0:HID] = raw[:, b, :]
            out[:, G[b], HID:] = raw[:, 32 + 7 - b, :]
    return out


class _Runner:
    """Caches the traced+compiled SPMD executable so repeat calls skip the
    (expensive) jax retrace and BIR re-serialization."""

    def __init__(self, S):
        import jax
        from jax.sharding import Mesh, PartitionSpec
        from jax.experimental.shard_map import shard_map
        from concourse import bass2jax
        from concourse.bass2jax import _bass_exec_p, partition_id_tensor

        bass2jax.install_neuronx_cc_hook()
        self.S = S
        nc = build_core_program(S)
        self.nc = nc
        partition_name = nc.partition_id_tensor.name if nc.partition_id_tensor else None
        in_names, out_names, out_avals, zero_outs = [], [], [], []
        for alloc in nc.m.functions[0].allocations:
            if not isinstance(alloc, mybir.MemoryLocationSet):
                continue
            name = alloc.memorylocations[0].name
            if alloc.kind == "ExternalInput":
                if name != partition_name:
                    in_names.append(name)
            elif alloc.kind == "ExternalOutput":
                shape = tuple(alloc.tensor_shape)
                dtype = mybir.dt.np(alloc.dtype)
                out_names.append(name)
                out_avals.append(jax.core.ShapedArray(shape, dtype))
                zero_outs.append(np.zeros(shape, dtype))
        n_params = len(in_names)
        self.in_names = list(in_names)
        self.out_names = out_names
        self.out_shapes = [tuple(a.shape) for a in out_avals]
        self.zero_outs = zero_outs
        all_in = in_names + out_names + ([partition_name] if partition_name else [])

        def _body(*args):
            operands = list(args)
            if partition_name is not None:
                operands.append(partition_id_tensor())
            return tuple(_bass_exec_p.bind(
                *operands,
                out_avals=tuple(out_avals),
                in_names=tuple(all_in),
                out_names=tuple(out_names),
                lowering_input_output_aliases=(),
                sim_require_finite=True,
                sim_require_nnan=True,
                nc=nc,
            ))

        devices = jax.devices()[:NCORES]
        mesh = Mesh(np.asarray(devices), ("core",))
        in_specs = (PartitionSpec("core"),) * (n_params + len(out_names))
        out_specs = (PartitionSpec("core"),) * len(out_names)
        self.fn = jax.jit(
            shard_map(_body, mesh=mesh, in_specs=in_specs,
                      out_specs=out_specs, check_rep=False),
            keep_unused=True)
        self.jax = jax

    def run(self, in_maps):
        concat_in = [
            np.concatenate([np.asarray(m[nm]) for m in in_maps], axis=0)
            for nm in self.in_names]
        concat_zero = [np.zeros((NCORES * z.shape[0], *z.shape[1:]), z.dtype)
                       for z in self.zero_outs]
        outs = self.fn(*concat_in, *concat_zero)
        return [
            {nm: np.asarray(outs[i]).reshape(NCORES, *self.out_shapes[i])[c]
             for i, nm in enumerate(self.out_names)}
            for c in range(NCORES)]

    def run_timed(self, in_maps, iters=5):
        """Stage inputs (and the pre-zeroed output buffers — every output
        element is written, so reuse is safe) on device; time executions."""
        import time
        concat_in = [
            self.jax.device_put(np.concatenate(
                [np.asarray(m[nm]) for m in in_maps], axis=0))
            for nm in self.in_names]
        concat_zero = [
            self.jax.device_put(
                np.zeros((NCORES * z.shape[0], *z.shape[1:]), z.dtype))
            for z in self.zero_outs]
        o = self.fn(*concat_in, *concat_zero)
        self.jax.block_until_ready(o)
        best = float("inf")
        for _ in range(iters):
            t0 = time.perf_counter()
            o = self.fn(*concat_in, *concat_zero)
            self.jax.block_until_ready(o)
            best = min(best, time.perf_counter() - t0)
        return best


def kernel(**inputs):
    S = inputs["x"].shape[0]
    if S not in _CACHE:
        _CACHE[S] = _Runner(S)
    runner = _CACHE[S]
    outs = runner.run(_in_maps(inputs))
    return _assemble(outs, S)


if __name__ == "__main__":
    rng = np.random.default_rng(0)
    S = 32
    inputs = {"x": rng.standard_normal((S, 64, 512), dtype=np.float32)}
    s = 1.0 / np.sqrt(HID)
    u = lambda *shp: rng.uniform(-s, s, shp).astype(np.float32)
    for c, idim in (("f0", 512), ("b0", 512), ("f1", 1024), ("b1", 1024)):
        inputs[f"wih_{c}"] = u(G3, idim)
        inputs[f"whh_{c}"] = u(G3, HID)
        inputs[f"bih_{c}"] = u(G3)
        inputs[f"bhh_{c}"] = u(G3)
    out = kernel(**inputs)
    print("kernel ran, out", out.shape, float(np.abs(out).mean()))



# revision 26
# speedup vs baseline: 49.9289x; 49.9289x over previous
"""Trainium2 Bass kernel for a 2-layer "BiGRU" (batch-flipped, per reference).

Design (one GRU cell per core, layer-pipelined over 8 cores):
  * The reference's "backward" direction flips the BATCH dim, not time. In
    hat-space (track hb_hat[b] := hb[B-1-b]) every cell consumes the
    UNFLIPPED input stream; the flip appears only in layer-1's input concat
    xi1[j] = [hf0[j], hb_hat0[flip j]] and in the final output assembly.
  * 8 cores = 4 cells x 2 flip-closed batch halves (32 samples each):
      core 0/1: f0 (A/B)   core 2/3: b0 (A/B)
      core 4/5: f1 (A/B)   core 6/7: b1 (A/B)
    Layer-0 cores stream transposed-h chunks to layer-1 cores with a
    per-chunk AllGather over groups {0,2,4,6} / {1,3,5,7}; layer 1 runs one
    chunk behind layer 0 (pipelined via tile deps on the collective).
  * Per-core recurrent work is one cell: 12 whh matmuls + 3 PSUM injects
    per step. Input-side matmuls (x@wihT / xi1@wih1T) are bulk-computed per
    chunk at full PE width from a shared 8-K-chunk lhs buffer (layer 0
    zero-pads chunks 4-7, so the batch-flip view applied to those chunks is
    a no-op) and bounced through DRAM to rotate rows into 32-partition ring
    windows.
  * Gate columns are permuted to [r0 z0 | r1 z1 | n0 n1] so each hidden
    half's gates are contiguous in PSUM and the halves pipeline. Uses
    zm = sigmoid(-z_preact) = 1-z so h' = zm*n + (h - zm*h) with no z
    sigmoid materialized.

Self-contained: hardcodes all shapes from the problem spec.
"""

import contextlib

import numpy as np

from concourse import bacc, tile
from concourse.bass import mybir

SEQ, BATCH, IN, HID = 512, 64, 512, 512
G3 = 3 * HID          # 1536
BC = 32               # batch per core
NCORES = 8
CH = 32               # steps per transfer chunk
W4 = 2                # gi ring window (steps)
FP32 = mybir.dt.float32
BF16 = mybir.dt.bfloat16

# gate-column permutation [r0 z0 | r1 z1 | n0 n1] (256 each)
_PERM = np.concatenate([np.arange(0, 256), np.arange(512, 768),
                        np.arange(256, 512), np.arange(768, 1024),
                        np.arange(1024, 1536)])


def _blob_layout():
    off = {}
    cur = 0
    for k in range(8):
        off[f"w{k}"] = cur; cur += G3          # wihT chunks (L0 uses 0-3)
    for k in range(4):
        off[f"u{k}"] = cur; cur += G3          # whhT chunks
    off["bulkb"] = cur; cur += G3              # row 0: gi bias (permuted)
    off["bhn"] = cur; cur += 512               # row 0: bhh n-part [n0 n1]
    off["ones"] = cur; cur += 128              # row 0: ones
    off["i32"] = cur; cur += 32                # rows 0:32 identity
    off["mcb"] = cur; cur += 128               # (128,128) carry mask (L0=1)
    off["mh"] = cur; cur += 512                # rows 0:32 carry mask (L0=1)
    return off, cur


def build_core_program(S, for_sim=False, role=None):
    """role=None: runtime If on partition id. role=0/1: branchless L0/L1
    variant (for single-core TimelineSim only)."""
    assert S % CH == 0 and CH % W4 == 0 and CH % 8 == 0
    NCHUNK = S // CH
    nc = bacc.Bacc(None, target_bir_lowering=False)
    off, totw = _blob_layout()
    blob_d = nc.declare_dram_parameter("blob", [128, totw // 2], FP32, isOutput=False)
    xTp_d = nc.declare_dram_parameter("xTp", [128, 4, S * BC // 2], FP32, isOutput=False)
    out_d = nc.declare_dram_parameter("out", [S, BC, HID], BF16, isOutput=True)

    ACT = mybir.ActivationFunctionType
    OP = mybir.AluOpType

    with tile.TileContext(nc) as tc:
        ctx = contextlib.ExitStack()
        with ctx:
            const = ctx.enter_context(tc.tile_pool(name="const", bufs=1))
            dram = ctx.enter_context(tc.tile_pool(name="dram", bufs=1, space="DRAM"))
            ghp = ctx.enter_context(tc.tile_pool(name="ghp", bufs=2, space="PSUM"))
            ptrp = ctx.enter_context(tc.tile_pool(name="ptrp", bufs=1, space="PSUM"))
            scr = ctx.enter_context(tc.tile_pool(name="scr", bufs=1, space="PSUM"))
            cb_pool = ctx.enter_context(tc.tile_pool(name="cb", bufs=2))
            ev_pool = ctx.enter_context(tc.tile_pool(name="ev", bufs=2))
            ring_pool = ctx.enter_context(tc.tile_pool(name="ring", bufs=3))
            hs_pool = ctx.enter_context(tc.tile_pool(name="hs", bufs=5))
            g_pool = ctx.enter_context(tc.tile_pool(name="g", bufs=2))

            # ---- constants ----
            blob = const.tile([128, totw // 2], FP32, tag="blob")
            nc.gpsimd.dma_start(out=blob[:], in_=blob_d[:])
            b16 = blob[:].bitcast(BF16)
            W = [b16[:, off[f"w{k}"]:off[f"w{k}"] + G3] for k in range(8)]
            U = [b16[:, off[f"u{k}"]:off[f"u{k}"] + G3] for k in range(4)]
            BULKB = b16[0:1, off["bulkb"]:off["bulkb"] + G3]
            BHN = b16[0:1, off["bhn"]:off["bhn"] + 512]
            ONES = b16[0:1, off["ones"]:off["ones"] + 128]
            IDT = b16[0:32, off["i32"]:off["i32"] + 32]
            MCB = b16[:, off["mcb"]:off["mcb"] + 128]
            MH = b16[0:BC, off["mh"]:off["mh"] + 512]

            zero16 = const.tile([128, 1024], BF16, tag="zero16")
            nc.any.memset(zero16[:], 0.0)

            # persistent double-buffered bulk-lhs: chunks 0-3 (x or hf0T) in
            # lhsbuf, chunks 4-7 (batch-flipped hb0T; zero on layer-0 cores)
            # in flipbuf.
            lhsbuf = [const.tile([128, 4, CH * BC // 2], FP32, tag=f"lhs{i}",
                                 name=f"lhs{i}") for i in range(2)]
            flipbuf = [const.tile([128, 4, CH * BC], BF16, tag=f"flip{i}",
                                  name=f"flip{i}") for i in range(2)]
            for i in range(2):
                nc.any.memset(lhsbuf[i][:], 0.0)
                nc.any.memset(flipbuf[i][:], 0.0)

            # ---- DRAM scratch ----
            gi_dram = [dram.tile([CH * BC, G3], BF16, tag="gi", bufs=4,
                                 name=f"gi{c}") for c in range(NCHUNK + 1)]
            send_dram = [dram.tile([128, 4, CH * BC], BF16, tag="send", bufs=2,
                                   name=f"send{c}") for c in range(NCHUNK)]
            ag_dram = [dram.tile([4, 128, 4, CH * BC], BF16, tag="agout", bufs=2,
                                 name=f"ag{c}") for c in range(NCHUNK)]
            # finite AllGather inputs on non-sender (L1) ranks
            for sl in range(min(2, NCHUNK)):
                for k in range(4):
                    nc.sync.dma_start(out=send_dram[sl][:, k, :],
                                      in_=zero16[:, 0:CH * BC])

            pid = nc.partition_id()

            cbufs = {}
            hstage = {}

            def bulk_chunk(c):
                lhs16 = lhsbuf[c % 2][:].bitcast(BF16)
                fb = flipbuf[c % 2]
                for m in range(CH * BC // 128):
                    ev = ev_pool.tile([128, G3], BF16, tag="ev")
                    for nb in range(3):
                        ps = scr.tile([128, 512], FP32, tag="scr", bufs=1,
                                      name=f"bs{c}_{m}_{nb}")
                        nc.tensor.matmul(out=ps[:], lhsT=ONES,
                                         rhs=BULKB[:, nb * 512:(nb + 1) * 512],
                                         start=True, stop=False)
                        for k in range(8):
                            if k < 4:
                                lhsT = lhs16[:, k, m * 128:(m + 1) * 128]
                            else:
                                lhsT = fb[:, k - 4, m * 128:(m + 1) * 128]
                            nc.tensor.matmul(out=ps[:], lhsT=lhsT,
                                             rhs=W[k][:, nb * 512:(nb + 1) * 512],
                                             start=False, stop=(k == 7))
                        nc.vector.tensor_copy(out=ev[:, nb * 512:(nb + 1) * 512],
                                              in_=ps[:])
                    nc.sync.dma_start(out=gi_dram[c][m * 128:(m + 1) * 128, :],
                                      in_=ev[:])

            def step(t, ring, wi):
                c, s = t // CH, t % CH
                if s == 0:
                    cbufs[c] = cb_pool.tile([128, 4, CH * BC], BF16, tag="cb",
                                            name=f"cb{c}")
                if t % 8 == 0:
                    hstage[t // 8] = hs_pool.tile([BC, 8, HID], BF16, tag="hs",
                                                  name=f"hs{t // 8}")
                hrow = hstage[t // 8][:, t % 8, :]
                hprev = (zero16[0:BC, 0:512] if t == 0
                         else hstage[(t - 1) // 8][:, (t - 1) % 8, :])
                mbb = None
                if t == CH:
                    # layer-1 cores' iteration 0 was a garbage warm-up chunk;
                    # zero their h carry here (mask is 1 on layer-0 cores).
                    mbb = g_pool.tile([128, 4, BC], BF16, tag="mbb", bufs=1)
                    nc.vector.tensor_tensor(
                        out=mbb[:],
                        in0=cbufs[0][:, :, (CH - 1) * BC:CH * BC],
                        in1=MCB.rearrange("p (k b) -> p k b", b=BC), op=OP.mult)
                    hpm = g_pool.tile([BC, 512], BF16, tag="hpm", bufs=1)
                    nc.vector.tensor_tensor(out=hpm[:], in0=hprev, in1=MH,
                                            op=OP.mult)
                    hprev = hpm[:]

                gh = ghp.tile([BC, G3], FP32, tag="gh", name=f"gh{t}")
                rg = ring[:, wi, :]
                for nb in (0, 2, 1):
                    if nb == 2:
                        nc.tensor.matmul(out=gh[:, 1024:1536], lhsT=ONES[:, 0:BC],
                                         rhs=BHN, start=True, stop=False)
                    else:
                        nc.tensor.matmul(out=gh[:, nb * 512:(nb + 1) * 512],
                                         lhsT=IDT,
                                         rhs=rg[:, nb * 512:(nb + 1) * 512],
                                         start=True, stop=False)
                    for k in range(4):
                        if t == 0:
                            lhsT = zero16[:, k * 32:k * 32 + BC]
                        elif t == CH:
                            lhsT = mbb[:, k, :]
                        else:
                            pc, psl = (t - 1) // CH, (t - 1) % CH
                            lhsT = cbufs[pc][:, k, psl * BC:(psl + 1) * BC]
                        nc.tensor.matmul(out=gh[:, nb * 512:(nb + 1) * 512],
                                         lhsT=lhsT,
                                         rhs=U[k][:, nb * 512:(nb + 1) * 512],
                                         start=False, stop=(k == 3))

                pt = ptrp.tile([128, 4, BC], BF16, tag="ptr", bufs=1,
                               name=f"ptr{t}")
                for hf in (0, 1):
                    rb = hf * 512
                    no = 1024 + hf * 256
                    sr = g_pool.tile([BC, 256], BF16, tag=f"sr{hf}")
                    zm = g_pool.tile([BC, 256], BF16, tag=f"zm{hf}")
                    u16 = g_pool.tile([BC, 256], BF16, tag=f"u{hf}")
                    v16 = g_pool.tile([BC, 256], BF16, tag=f"v{hf}")
                    n16 = g_pool.tile([BC, 256], BF16, tag=f"n{hf}")
                    m16 = g_pool.tile([BC, 256], BF16, tag=f"m{hf}")
                    mh = g_pool.tile([BC, 256], BF16, tag=f"mh{hf}")
                    d16 = g_pool.tile([BC, 256], BF16, tag=f"d{hf}")
                    nc.scalar.activation(sr[:], gh[:, rb:rb + 256], ACT.Sigmoid)
                    nc.scalar.activation(zm[:], gh[:, rb + 256:rb + 512],
                                         ACT.Sigmoid, scale=-1.0)
                    nc.vector.tensor_tensor(out=u16[:], in0=sr[:],
                                            in1=gh[:, no:no + 256], op=OP.mult)
                    nc.vector.tensor_tensor(out=v16[:], in0=u16[:],
                                            in1=rg[:, no:no + 256], op=OP.add)
                    nc.scalar.activation(n16[:], v16[:], ACT.Tanh)
                    hp = hprev[:, hf * 256:(hf + 1) * 256]
                    nc.vector.tensor_tensor(out=mh[:], in0=zm[:], in1=hp,
                                            op=OP.mult)
                    nc.vector.tensor_tensor(out=d16[:], in0=hp, in1=mh[:],
                                            op=OP.subtract)
                    nc.vector.tensor_tensor(out=m16[:], in0=zm[:], in1=n16[:],
                                            op=OP.mult)
                    hout = hrow[:, hf * 256:(hf + 1) * 256]
                    nc.vector.tensor_tensor(out=hout, in0=m16[:], in1=d16[:],
                                            op=OP.add)
                    for kk in (0, 1):
                        nc.tensor.transpose(out=pt[:, 2 * hf + kk, :],
                                            in_=hout[:, kk * 128:(kk + 1) * 128],
                                            identity=IDT)
                    nc.vector.tensor_copy(
                        out=cbufs[c][:, 2 * hf:2 * hf + 2, s * BC:(s + 1) * BC],
                        in_=pt[:, 2 * hf:2 * hf + 2, :])

            # ---- chunk pipeline (iteration i: L0 computes chunk i, L1's
            # shared instructions compute its real chunk i-1 from AG_{i-1};
            # L1's iteration 0 is a warm-up on zero inputs, masked off at
            # t==CH; L0's last iteration is a discarded tail). ----
            for i in range(NCHUNK + 1):
                cw = CH * BC // 2
                if i < NCHUNK and role in (None, 0):
                    cm = (tc.If(pid < 4) if role is None
                          else contextlib.nullcontext())
                    with cm:
                        nc.sync.dma_start(out=lhsbuf[i % 2][:, 0:4, :],
                                          in_=xTp_d[:, :, i * cw:(i + 1) * cw])
                if i > 0 and role in (None, 1):
                    cm = (tc.If(pid >= 4) if role is None
                          else contextlib.nullcontext())
                    with cm:
                        v = lhsbuf[i % 2][:].bitcast(BF16)
                        nc.sync.dma_start(out=v[:, 0:4, :], in_=ag_dram[i - 1][0])
                        agt = ev_pool.tile([128, 4, CH * BC], BF16, tag="agt",
                                           name=f"agt{i}")
                        nc.sync.dma_start(out=agt[:], in_=ag_dram[i - 1][1])
                        # batch-flip within each step block (receiver side)
                        nc.vector.tensor_copy(
                            out=flipbuf[i % 2][:].rearrange(
                                "p k (s b) -> p (k s) b", b=BC),
                            in_=agt[:].rearrange(
                                "p k (s b) -> p (k s) b", b=BC)[:, :, ::-1])
                bulk_chunk(i)
                for w in range(CH // W4):
                    r = ring_pool.tile([BC, W4, G3], BF16, tag="ring",
                                       name=f"ring{i}_{w}")
                    src = gi_dram[i][:].rearrange("(s b) g -> b s g", b=BC)
                    nc.sync.dma_start(out=r[:], in_=src[:, w * W4:(w + 1) * W4, :])
                    for j in range(W4):
                        step(i * CH + w * W4 + j, r, j)
                if i < NCHUNK:
                    # L0-only send: L1's AllGather input is the setup-time
                    # zero buffer, so its AG_i has no dependency on its own
                    # iteration-i compute and overlaps it.
                    if role in (None, 0):
                        cm = (tc.If(pid < 4) if role is None
                              else contextlib.nullcontext())
                        with cm:
                            nc.sync.dma_start(out=send_dram[i][:],
                                              in_=cbufs[i][:])
                    nc.gpsimd.collective_compute(
                        "AllGather", mybir.AluOpType.bypass,
                        replica_groups=[[0, 2, 4, 6], [1, 3, 5, 7]],
                        ins=[send_dram[i].opt()], outs=[ag_dram[i].opt()])
                if i > 0 and role in (None, 1):
                    cm = (tc.If(pid >= 4) if role is None
                          else contextlib.nullcontext())
                    with cm:
                        for blk in range(CH // 8):
                            nc.sync.dma_start(
                                out=out_d[:].rearrange(
                                    "(a e) b h -> b a e h", e=8)[:, (i - 1) * (CH // 8) + blk],
                                in_=hstage[i * (CH // 8) + blk][:])
    nc.compile()
    if for_sim:
        nc.insert_bir_kernel_barrier_sem_inc()
    return nc


# ---------------------------------------------------------------------------
# host side
# ---------------------------------------------------------------------------

_CACHE = {}

# batch sets: A = rows 0..15 + 48..63, B = rows 16..47 (both flip-closed,
# local flip = reversal of the 32 local rows)
_SET_A = list(range(16)) + list(range(48, 64))
_SET_B = list(range(16, 48))


def _bf16_u16(a):
    a = np.ascontiguousarray(a, np.float32)
    u = a.view(np.uint32)
    return ((u + 0x7FFF + ((u >> 16) & 1)) >> 16).astype(np.uint16)


def _pack_words(u16):
    ev = u16[..., 0::2].astype(np.uint32)
    od = u16[..., 1::2].astype(np.uint32)
    return (ev | (od << 16)).view(np.float32)


def _blob_host(wih, whh, bih, bhh, carry):
    """Per-core constant blob for one cell. wih: (1536, K) with K in
    {512, 1024}; columns of wihT/whhT permuted to [r0 z0 r1 z1 n0 n1]."""
    off, totw = _blob_layout()
    blob = np.zeros((128, totw), np.uint16)
    K = wih.shape[1]
    wt = wih.T[:, _PERM]                      # (K, 1536) permuted
    kx = K // 128
    wt = _bf16_u16(wt.reshape(kx, 128, G3))
    for k in range(kx):
        blob[:, off[f"w{k}"]:off[f"w{k}"] + G3] = wt[k]
    ut = _bf16_u16(whh.T[:, _PERM].reshape(4, 128, G3))
    for k in range(4):
        blob[:, off[f"u{k}"]:off[f"u{k}"] + G3] = ut[k]
    bulkb = np.concatenate([(bih + bhh)[:1024], bih[1024:]])[_PERM]
    blob[0, off["bulkb"]:off["bulkb"] + G3] = _bf16_u16(bulkb)
    blob[0, off["bhn"]:off["bhn"] + 512] = _bf16_u16(bhh[1024:])
    one = _bf16_u16(np.ones(1, np.float32))[0]
    blob[0, off["ones"]:off["ones"] + 128] = one
    for j in range(32):
        blob[j, off["i32"] + j] = one
    if carry:
        blob[:, off["mcb"]:off["mcb"] + 128] = one
        blob[0:32, off["mh"]:off["mh"] + 512] = one
    return _pack_words(blob)


def _in_maps(inputs):
    S = inputs["x"].shape[0]
    x = np.asarray(inputs["x"], np.float32)
    cells = [("f0", _SET_A), ("f0", _SET_B), ("b0", _SET_A), ("b0", _SET_B),
             ("f1", _SET_A), ("f1", _SET_B), ("b1", _SET_A), ("b1", _SET_B)]
    in_maps = []
    zx = np.zeros((128, 4, S * BC // 2), np.float32)
    for d in range(NCORES):
        cname, bset = cells[d]
        blob = _blob_host(np.asarray(inputs[f"wih_{cname}"], np.float32),
                          np.asarray(inputs[f"whh_{cname}"], np.float32),
                          np.asarray(inputs[f"bih_{cname}"], np.float32),
                          np.asarray(inputs[f"bhh_{cname}"], np.float32),
                          carry=(d < 4))
        if d < 4:
            xl = x[:, bset, :]                               # (S, 32, 512)
            xT = _bf16_u16(xl.transpose(2, 0, 1).reshape(4, 128, S * BC))
            xT = np.ascontiguousarray(xT.transpose(1, 0, 2))  # (128,4,S*BC)
            xw = _pack_words(xT)
        else:
            xw = zx
        in_maps.append({"blob": blob, "xTp": xw})
    return in_maps


def _assemble(outs, S):
    out = np.zeros((S, BATCH, 2 * HID), np.float32)
    for gi_, bset in ((0, _SET_A), (1, _SET_B)):
        f1 = np.asarray(outs[4 + gi_]["out"], np.float32)    # (S, 32, 512)
        b1 = np.asarray(outs[6 + gi_]["out"], np.float32)
        for j, b in enumerate(bset):
            out[:, b, 0:HID] = f1[:, j, :]
            out[:, b, HID:] = b1[:, 31 - j, :]
    return out


class _Runner:
    """Caches the traced+compiled SPMD executable."""

    def __init__(self, S):
        import jax
        from jax.sharding import Mesh, PartitionSpec
        from jax.experimental.shard_map import shard_map
        from concourse import bass2jax
        from concourse.bass2jax import _bass_exec_p, partition_id_tensor

        bass2jax.install_neuronx_cc_hook()
        self.S = S
        nc = build_core_program(S)
        self.nc = nc
        partition_name = nc.partition_id_tensor.name if nc.partition_id_tensor else None
        in_names, out_names, out_avals, zero_outs = [], [], [], []
        for alloc in nc.m.functions[0].allocations:
            if not isinstance(alloc, mybir.MemoryLocationSet):
                continue
            name = alloc.memorylocations[0].name
            if alloc.kind == "ExternalInput":
                if name != partition_name:
                    in_names.append(name)
            elif alloc.kind == "ExternalOutput":
                shape = tuple(alloc.tensor_shape)
                dtype = mybir.dt.np(alloc.dtype)
                out_names.append(name)
                out_avals.append(jax.core.ShapedArray(shape, dtype))
                zero_outs.append(np.zeros(shape, dtype))
        n_params = len(in_names)
        self.in_names = list(in_names)
        self.out_names = out_names
        self.out_shapes = [tuple(a.shape) for a in out_avals]
        self.zero_outs = zero_outs
        all_in = in_names + out_names + ([partition_name] if partition_name else [])

        def _body(*args):
            operands = list(args)
            if partition_name is not None:
                operands.append(partition_id_tensor())
            return tuple(_bass_exec_p.bind(
                *operands,
                out_avals=tuple(out_avals),
                in_names=tuple(all_in),
                out_names=tuple(out_names),
                lowering_input_output_aliases=(),
                sim_require_finite=True,
                sim_require_nnan=True,
                nc=nc,
            ))

        devices = jax.devices()[:NCORES]
        mesh = Mesh(np.asarray(devices), ("core",))
        in_specs = (PartitionSpec("core"),) * (n_params + len(out_names))
        out_specs = (PartitionSpec("core"),) * len(out_names)
        self.fn = jax.jit(
            shard_map(_body, mesh=mesh, in_specs=in_specs,
                      out_specs=out_specs, check_rep=False),
            keep_unused=True)
        self.jax = jax

    def run(self, in_maps):
        concat_in = [
            np.concatenate([np.asarray(m[nm]) for m in in_maps], axis=0)
            for nm in self.in_names]
        concat_zero = [np.zeros((NCORES * z.shape[0], *z.shape[1:]), z.dtype)
                       for z in self.zero_outs]
        outs = self.fn(*concat_in, *concat_zero)
        return [
            {nm: np.asarray(outs[i]).reshape(NCORES, *self.out_shapes[i])[c]
             for i, nm in enumerate(self.out_names)}
            for c in range(NCORES)]

    def run_timed(self, in_maps, iters=5):
        import time
        concat_in = [
            self.jax.device_put(np.concatenate(
                [np.asarray(m[nm]) for m in in_maps], axis=0))
            for nm in self.in_names]
        concat_zero = [
            self.jax.device_put(
                np.zeros((NCORES * z.shape[0], *z.shape[1:]), z.dtype))
            for z in self.zero_outs]
        o = self.fn(*concat_in, *concat_zero)
        self.jax.block_until_ready(o)
        best = float("inf")
        for _ in range(iters):
            t0 = time.perf_counter()
            o = self.fn(*concat_in, *concat_zero)
            self.jax.block_until_ready(o)
            best = min(best, time.perf_counter() - t0)
        return best


def kernel(**inputs):
    S = inputs["x"].shape[0]
    if S not in _CACHE:
        _CACHE[S] = _Runner(S)
    runner = _CACHE[S]
    outs = runner.run(_in_maps(inputs))
    return _assemble(outs, S)


if __name__ == "__main__":
    rng = np.random.default_rng(0)
    S = 32
    inputs = {"x": rng.standard_normal((S, 64, 512), dtype=np.float32)}
    s = 1.0 / np.sqrt(HID)
    u = lambda *shp: rng.uniform(-s, s, shp).astype(np.float32)
    for c, idim in (("f0", 512), ("b0", 512), ("f1", 1024), ("b1", 1024)):
        inputs[f"wih_{c}"] = u(G3, idim)
        inputs[f"whh_{c}"] = u(G3, HID)
        inputs[f"bih_{c}"] = u(G3)
        inputs[f"bhh_{c}"] = u(G3)
    out = kernel(**inputs)
    print("kernel ran, out", out.shape, float(np.abs(out).mean()))


# revision 31
# speedup vs baseline: 51.6514x; 1.0345x over previous
"""Trainium2 Bass kernel for a 2-layer "BiGRU" (batch-flipped, per reference).

Design (one GRU cell per core, layer-pipelined over 8 cores):
  * The reference's "backward" direction flips the BATCH dim, not time. In
    hat-space (track hb_hat[b] := hb[B-1-b]) every cell consumes the
    UNFLIPPED input stream; the flip appears only in layer-1's input concat
    xi1[j] = [hf0[j], hb_hat0[flip j]] and in the final output assembly.
  * 8 cores = 4 cells x 2 flip-closed batch halves (32 samples each):
      core 0/1: f0 (A/B)   core 2/3: b0 (A/B)
      core 4/5: f1 (A/B)   core 6/7: b1 (A/B)
    Layer-0 cores stream transposed-h chunks to layer-1 cores with a
    per-chunk AllGather over groups {0,2,4,6} / {1,3,5,7}; layer 1 runs one
    chunk behind layer 0 (pipelined via tile deps on the collective).
  * Per-core recurrent work is one cell: 12 whh matmuls + 3 PSUM injects
    per step. Input-side matmuls (x@wihT / xi1@wih1T) are bulk-computed per
    chunk at full PE width from a shared 8-K-chunk lhs buffer (layer 0
    zero-pads chunks 4-7, so the batch-flip view applied to those chunks is
    a no-op) and bounced through DRAM to rotate rows into 32-partition ring
    windows.
  * Gate columns are permuted to [r0 z0 | r1 z1 | n0 n1] so each hidden
    half's gates are contiguous in PSUM and the halves pipeline. Uses
    zm = sigmoid(-z_preact) = 1-z so h' = zm*n + (h - zm*h) with no z
    sigmoid materialized.

Self-contained: hardcodes all shapes from the problem spec.
"""

import contextlib

import numpy as np

from concourse import bacc, tile
from concourse.bass import mybir

SEQ, BATCH, IN, HID = 512, 64, 512, 512
G3 = 3 * HID          # 1536
BC = 32               # batch per core
NCORES = 8
CH = 32               # steps per transfer chunk
W4 = 2                # gi ring window (steps)
FP32 = mybir.dt.float32
BF16 = mybir.dt.bfloat16

# gate-column permutation [r0 z0 | r1 z1 | n0 n1] (256 each)
_PERM = np.concatenate([np.arange(0, 256), np.arange(512, 768),
                        np.arange(256, 512), np.arange(768, 1024),
                        np.arange(1024, 1536)])


def _blob_layout():
    off = {}
    cur = 0
    for k in range(8):
        off[f"w{k}"] = cur; cur += G3          # wihT chunks (L0 uses 0-3)
    for k in range(4):
        off[f"u{k}"] = cur; cur += G3          # whhT chunks
    off["bulkb"] = cur; cur += G3              # row 0: gi bias (permuted)
    off["bhn"] = cur; cur += 512               # row 0: bhh n-part [n0 n1]
    off["ones"] = cur; cur += 128              # row 0: ones
    off["i32"] = cur; cur += 32                # rows 0:32 identity
    off["mcb"] = cur; cur += 128               # (128,128) carry mask (L0=1)
    off["mh"] = cur; cur += 512                # rows 0:32 carry mask (L0=1)
    return off, cur


def build_core_program(S, for_sim=False, role=None):
    """role=None: runtime If on partition id. role=0/1: branchless L0/L1
    variant (for single-core TimelineSim only)."""
    assert S % CH == 0 and CH % W4 == 0 and CH % 8 == 0
    NCHUNK = S // CH
    nc = bacc.Bacc(None, target_bir_lowering=False)
    off, totw = _blob_layout()
    blob_d = nc.declare_dram_parameter("blob", [128, totw // 2], FP32, isOutput=False)
    xTp_d = nc.declare_dram_parameter("xTp", [128, 4, S * BC // 2], FP32, isOutput=False)
    out_d = nc.declare_dram_parameter("out", [S, BC, HID], BF16, isOutput=True)

    ACT = mybir.ActivationFunctionType
    OP = mybir.AluOpType

    with tile.TileContext(nc) as tc:
        ctx = contextlib.ExitStack()
        with ctx:
            const = ctx.enter_context(tc.tile_pool(name="const", bufs=1))
            dram = ctx.enter_context(tc.tile_pool(name="dram", bufs=1, space="DRAM"))
            ghp = ctx.enter_context(tc.tile_pool(name="ghp", bufs=2, space="PSUM"))
            ptrp = ctx.enter_context(tc.tile_pool(name="ptrp", bufs=1, space="PSUM"))
            scr = ctx.enter_context(tc.tile_pool(name="scr", bufs=1, space="PSUM"))
            cb_pool = ctx.enter_context(tc.tile_pool(name="cb", bufs=2))
            ev_pool = ctx.enter_context(tc.tile_pool(name="ev", bufs=2))
            ring_pool = ctx.enter_context(tc.tile_pool(name="ring", bufs=3))
            hs_pool = ctx.enter_context(tc.tile_pool(name="hs", bufs=5))
            g_pool = ctx.enter_context(tc.tile_pool(name="g", bufs=2))

            # ---- constants ----
            blob = const.tile([128, totw // 2], FP32, tag="blob")
            nc.gpsimd.dma_start(out=blob[:], in_=blob_d[:])
            b16 = blob[:].bitcast(BF16)
            W = [b16[:, off[f"w{k}"]:off[f"w{k}"] + G3] for k in range(8)]
            U = [b16[:, off[f"u{k}"]:off[f"u{k}"] + G3] for k in range(4)]
            BULKB = b16[0:1, off["bulkb"]:off["bulkb"] + G3]
            BHN = b16[0:1, off["bhn"]:off["bhn"] + 512]
            ONES = b16[0:1, off["ones"]:off["ones"] + 128]
            IDT = b16[0:32, off["i32"]:off["i32"] + 32]
            MCB = b16[:, off["mcb"]:off["mcb"] + 128]
            MH = b16[0:BC, off["mh"]:off["mh"] + 512]

            zero16 = const.tile([128, 1024], BF16, tag="zero16")
            nc.any.memset(zero16[:], 0.0)

            # persistent double-buffered bulk-lhs: chunks 0-3 (x or hf0T) in
            # lhsbuf, chunks 4-7 (batch-flipped hb0T; zero on layer-0 cores)
            # in flipbuf.
            lhsbuf = [const.tile([128, 4, CH * BC // 2], FP32, tag=f"lhs{i}",
                                 name=f"lhs{i}") for i in range(2)]
            flipbuf = [const.tile([128, 4, CH * BC], BF16, tag=f"flip{i}",
                                  name=f"flip{i}") for i in range(2)]
            for i in range(2):
                nc.any.memset(lhsbuf[i][:], 0.0)
                nc.any.memset(flipbuf[i][:], 0.0)

            # ---- DRAM scratch ----
            gi_dram = [dram.tile([CH * BC, G3], BF16, tag="gi", bufs=4,
                                 name=f"gi{c}") for c in range(NCHUNK + 1)]
            send_dram = [dram.tile([128, 4, CH * BC], BF16, tag="send", bufs=2,
                                   name=f"send{c}") for c in range(NCHUNK)]
            ag_dram = [dram.tile([4, 128, 4, CH * BC], BF16, tag="agout", bufs=2,
                                 name=f"ag{c}") for c in range(NCHUNK)]
            # finite AllGather inputs on non-sender (L1) ranks
            for sl in range(min(2, NCHUNK)):
                for k in range(4):
                    nc.sync.dma_start(out=send_dram[sl][:, k, :],
                                      in_=zero16[:, 0:CH * BC])

            pid = nc.partition_id()

            cbufs = {}
            hstage = {}

            def bulk_chunk(c):
                lhs16 = lhsbuf[c % 2][:].bitcast(BF16)
                fb = flipbuf[c % 2]
                for m in range(CH * BC // 128):
                    ev = ev_pool.tile([128, G3], BF16, tag="ev")
                    for nb in range(3):
                        ps = scr.tile([128, 512], FP32, tag="scr", bufs=1,
                                      name=f"bs{c}_{m}_{nb}")
                        nc.tensor.matmul(out=ps[:], lhsT=ONES,
                                         rhs=BULKB[:, nb * 512:(nb + 1) * 512],
                                         start=True, stop=False)
                        for k in range(8):
                            if k < 4:
                                lhsT = lhs16[:, k, m * 128:(m + 1) * 128]
                            else:
                                lhsT = fb[:, k - 4, m * 128:(m + 1) * 128]
                            nc.tensor.matmul(out=ps[:], lhsT=lhsT,
                                             rhs=W[k][:, nb * 512:(nb + 1) * 512],
                                             start=False, stop=(k == 7))
                        nc.vector.tensor_copy(out=ev[:, nb * 512:(nb + 1) * 512],
                                              in_=ps[:])
                    nc.gpsimd.dma_start(out=gi_dram[c][m * 128:(m + 1) * 128, :],
                                        in_=ev[:])

            def step(t, ring, wi):
                c, s = t // CH, t % CH
                if s == 0:
                    cbufs[c] = cb_pool.tile([128, 4, CH * BC], BF16, tag="cb",
                                            name=f"cb{c}")
                if t % 8 == 0:
                    hstage[t // 8] = hs_pool.tile([BC, 8, HID], BF16, tag="hs",
                                                  name=f"hs{t // 8}")
                hrow = hstage[t // 8][:, t % 8, :]
                hprev = (zero16[0:BC, 0:512] if t == 0
                         else hstage[(t - 1) // 8][:, (t - 1) % 8, :])
                mbb = None
                if t == CH:
                    # layer-1 cores' iteration 0 was a garbage warm-up chunk;
                    # zero their h carry here (mask is 1 on layer-0 cores).
                    mbb = g_pool.tile([128, 4, BC], BF16, tag="mbb", bufs=1)
                    nc.vector.tensor_tensor(
                        out=mbb[:],
                        in0=cbufs[0][:, :, (CH - 1) * BC:CH * BC],
                        in1=MCB.rearrange("p (k b) -> p k b", b=BC), op=OP.mult)
                    hpm = g_pool.tile([BC, 512], BF16, tag="hpm", bufs=1)
                    nc.vector.tensor_tensor(out=hpm[:], in0=hprev, in1=MH,
                                            op=OP.mult)
                    hprev = hpm[:]

                gh = ghp.tile([BC, G3], FP32, tag="gh", name=f"gh{t}")
                rg = ring[:, wi, :]
                for nb in (0, 2, 1):
                    if nb == 2:
                        nc.tensor.matmul(out=gh[:, 1024:1536], lhsT=ONES[:, 0:BC],
                                         rhs=BHN, start=True, stop=False)
                    else:
                        nc.tensor.matmul(out=gh[:, nb * 512:(nb + 1) * 512],
                                         lhsT=IDT,
                                         rhs=rg[:, nb * 512:(nb + 1) * 512],
                                         start=True, stop=False)
                    for k in range(4):
                        if t == 0:
                            lhsT = zero16[:, k * 32:k * 32 + BC]
                        elif t == CH:
                            lhsT = mbb[:, k, :]
                        else:
                            pc, psl = (t - 1) // CH, (t - 1) % CH
                            lhsT = cbufs[pc][:, k, psl * BC:(psl + 1) * BC]
                        nc.tensor.matmul(out=gh[:, nb * 512:(nb + 1) * 512],
                                         lhsT=lhsT,
                                         rhs=U[k][:, nb * 512:(nb + 1) * 512],
                                         start=False, stop=(k == 3))

                pt = ptrp.tile([128, 4, BC], BF16, tag="ptr", bufs=1,
                               name=f"ptr{t}")
                for hf in (0, 1):
                    rb = hf * 512
                    no = 1024 + hf * 256
                    sr = g_pool.tile([BC, 256], BF16, tag=f"sr{hf}")
                    zm = g_pool.tile([BC, 256], BF16, tag=f"zm{hf}")
                    u16 = g_pool.tile([BC, 256], BF16, tag=f"u{hf}")
                    v16 = g_pool.tile([BC, 256], BF16, tag=f"v{hf}")
                    n16 = g_pool.tile([BC, 256], BF16, tag=f"n{hf}")
                    m16 = g_pool.tile([BC, 256], BF16, tag=f"m{hf}")
                    mh = g_pool.tile([BC, 256], BF16, tag=f"mh{hf}")
                    d16 = g_pool.tile([BC, 256], BF16, tag=f"d{hf}")
                    nc.scalar.activation(sr[:], gh[:, rb:rb + 256], ACT.Sigmoid)
                    nc.scalar.activation(zm[:], gh[:, rb + 256:rb + 512],
                                         ACT.Sigmoid, scale=-1.0)
                    nc.vector.tensor_tensor(out=u16[:], in0=sr[:],
                                            in1=gh[:, no:no + 256], op=OP.mult)
                    nc.vector.tensor_tensor(out=v16[:], in0=u16[:],
                                            in1=rg[:, no:no + 256], op=OP.add)
                    nc.scalar.activation(n16[:], v16[:], ACT.Tanh)
                    hp = hprev[:, hf * 256:(hf + 1) * 256]
                    nc.vector.tensor_tensor(out=mh[:], in0=zm[:], in1=hp,
                                            op=OP.mult)
                    nc.vector.tensor_tensor(out=d16[:], in0=hp, in1=mh[:],
                                            op=OP.subtract)
                    nc.vector.tensor_tensor(out=m16[:], in0=zm[:], in1=n16[:],
                                            op=OP.mult)
                    hout = hrow[:, hf * 256:(hf + 1) * 256]
                    nc.vector.tensor_tensor(out=hout, in0=m16[:], in1=d16[:],
                                            op=OP.add)
                    for kk in (0, 1):
                        nc.tensor.transpose(out=pt[:, 2 * hf + kk, :],
                                            in_=hout[:, kk * 128:(kk + 1) * 128],
                                            identity=IDT)
                    nc.vector.tensor_copy(
                        out=cbufs[c][:, 2 * hf:2 * hf + 2, s * BC:(s + 1) * BC],
                        in_=pt[:, 2 * hf:2 * hf + 2, :])

            # ---- chunk pipeline (iteration i: L0 computes chunk i, L1's
            # shared instructions compute its real chunk i-1 from AG_{i-1};
            # L1's iteration 0 is a warm-up on zero inputs, masked off at
            # t==CH; L0's last iteration is a discarded tail). ----
            for i in range(NCHUNK + 1):
                cw = CH * BC // 2
                if i < NCHUNK and role in (None, 0):
                    cm = (tc.If(pid < 4) if role is None
                          else contextlib.nullcontext())
                    with cm:
                        nc.sync.dma_start(out=lhsbuf[i % 2][:, 0:4, :],
                                          in_=xTp_d[:, :, i * cw:(i + 1) * cw])
                if i > 0 and role in (None, 1):
                    cm = (tc.If(pid >= 4) if role is None
                          else contextlib.nullcontext())
                    with cm:
                        v = lhsbuf[i % 2][:].bitcast(BF16)
                        nc.sync.dma_start(out=v[:, 0:4, :], in_=ag_dram[i - 1][0])
                        agt = ev_pool.tile([128, 4, CH * BC], BF16, tag="agt",
                                           name=f"agt{i}")
                        nc.sync.dma_start(out=agt[:], in_=ag_dram[i - 1][1])
                        # batch-flip within each step block (receiver side)
                        nc.vector.tensor_copy(
                            out=flipbuf[i % 2][:].rearrange(
                                "p k (s b) -> p (k s) b", b=BC),
                            in_=agt[:].rearrange(
                                "p k (s b) -> p (k s) b", b=BC)[:, :, ::-1])
                bulk_chunk(i)
                for w in range(CH // W4):
                    r = ring_pool.tile([BC, W4, G3], BF16, tag="ring",
                                       name=f"ring{i}_{w}")
                    src = gi_dram[i][:].rearrange("(s b) g -> b s g", b=BC)
                    nc.sync.dma_start(out=r[:], in_=src[:, w * W4:(w + 1) * W4, :])
                    for j in range(W4):
                        step(i * CH + w * W4 + j, r, j)
                if i < NCHUNK:
                    # L0-only send: L1's AllGather input is the setup-time
                    # zero buffer, so its AG_i has no dependency on its own
                    # iteration-i compute and overlaps it.
                    if role in (None, 0):
                        cm = (tc.If(pid < 4) if role is None
                              else contextlib.nullcontext())
                        with cm:
                            nc.sync.dma_start(out=send_dram[i][:],
                                              in_=cbufs[i][:])
                    nc.gpsimd.collective_compute(
                        "AllGather", mybir.AluOpType.bypass,
                        replica_groups=[[0, 2, 4, 6], [1, 3, 5, 7]],
                        ins=[send_dram[i].opt()], outs=[ag_dram[i].opt()])
                if i > 0 and role in (None, 1):
                    cm = (tc.If(pid >= 4) if role is None
                          else contextlib.nullcontext())
                    with cm:
                        for blk in range(CH // 8):
                            nc.gpsimd.dma_start(
                                out=out_d[:].rearrange(
                                    "(a e) b h -> b a e h", e=8)[:, (i - 1) * (CH // 8) + blk],
                                in_=hstage[i * (CH // 8) + blk][:])
    nc.compile()
    if for_sim:
        nc.insert_bir_kernel_barrier_sem_inc()
    return nc


# ---------------------------------------------------------------------------
# host side
# ---------------------------------------------------------------------------

_CACHE = {}

# batch sets: A = rows 0..15 + 48..63, B = rows 16..47 (both flip-closed,
# local flip = reversal of the 32 local rows)
_SET_A = list(range(16)) + list(range(48, 64))
_SET_B = list(range(16, 48))


def _bf16_u16(a):
    a = np.ascontiguousarray(a, np.float32)
    u = a.view(np.uint32)
    return ((u + 0x7FFF + ((u >> 16) & 1)) >> 16).astype(np.uint16)


def _pack_words(u16):
    ev = u16[..., 0::2].astype(np.uint32)
    od = u16[..., 1::2].astype(np.uint32)
    return (ev | (od << 16)).view(np.float32)


def _blob_host(wih, whh, bih, bhh, carry):
    """Per-core constant blob for one cell. wih: (1536, K) with K in
    {512, 1024}; columns of wihT/whhT permuted to [r0 z0 r1 z1 n0 n1]."""
    off, totw = _blob_layout()
    blob = np.zeros((128, totw), np.uint16)
    K = wih.shape[1]
    wt = wih.T[:, _PERM]                      # (K, 1536) permuted
    kx = K // 128
    wt = _bf16_u16(wt.reshape(kx, 128, G3))
    for k in range(kx):
        blob[:, off[f"w{k}"]:off[f"w{k}"] + G3] = wt[k]
    ut = _bf16_u16(whh.T[:, _PERM].reshape(4, 128, G3))
    for k in range(4):
        blob[:, off[f"u{k}"]:off[f"u{k}"] + G3] = ut[k]
    bulkb = np.concatenate([(bih + bhh)[:1024], bih[1024:]])[_PERM]
    blob[0, off["bulkb"]:off["bulkb"] + G3] = _bf16_u16(bulkb)
    blob[0, off["bhn"]:off["bhn"] + 512] = _bf16_u16(bhh[1024:])
    one = _bf16_u16(np.ones(1, np.float32))[0]
    blob[0, off["ones"]:off["ones"] + 128] = one
    for j in range(32):
        blob[j, off["i32"] + j] = one
    if carry:
        blob[:, off["mcb"]:off["mcb"] + 128] = one
        blob[0:32, off["mh"]:off["mh"] + 512] = one
    return _pack_words(blob)


def _in_maps(inputs):
    S = inputs["x"].shape[0]
    x = np.asarray(inputs["x"], np.float32)
    cells = [("f0", _SET_A), ("f0", _SET_B), ("b0", _SET_A), ("b0", _SET_B),
             ("f1", _SET_A), ("f1", _SET_B), ("b1", _SET_A), ("b1", _SET_B)]
    in_maps = []
    zx = np.zeros((128, 4, S * BC // 2), np.float32)
    for d in range(NCORES):
        cname, bset = cells[d]
        blob = _blob_host(np.asarray(inputs[f"wih_{cname}"], np.float32),
                          np.asarray(inputs[f"whh_{cname}"], np.float32),
                          np.asarray(inputs[f"bih_{cname}"], np.float32),
                          np.asarray(inputs[f"bhh_{cname}"], np.float32),
                          carry=(d < 4))
        if d < 4:
            xl = x[:, bset, :]                               # (S, 32, 512)
            xT = _bf16_u16(xl.transpose(2, 0, 1).reshape(4, 128, S * BC))
            xT = np.ascontiguousarray(xT.transpose(1, 0, 2))  # (128,4,S*BC)
            xw = _pack_words(xT)
        else:
            xw = zx
        in_maps.append({"blob": blob, "xTp": xw})
    return in_maps


def _assemble(outs, S):
    out = np.zeros((S, BATCH, 2 * HID), np.float32)
    for gi_, bset in ((0, _SET_A), (1, _SET_B)):
        f1 = np.asarray(outs[4 + gi_]["out"], np.float32)    # (S, 32, 512)
        b1 = np.asarray(outs[6 + gi_]["out"], np.float32)
        for j, b in enumerate(bset):
            out[:, b, 0:HID] = f1[:, j, :]
            out[:, b, HID:] = b1[:, 31 - j, :]
    return out


class _Runner:
    """Caches the traced+compiled SPMD executable."""

    def __init__(self, S):
        import jax
        from jax.sharding import Mesh, PartitionSpec
        from jax.experimental.shard_map import shard_map
        from concourse import bass2jax
        from concourse.bass2jax import _bass_exec_p, partition_id_tensor

        bass2jax.install_neuronx_cc_hook()
        self.S = S
        nc = build_core_program(S)
        self.nc = nc
        partition_name = nc.partition_id_tensor.name if nc.partition_id_tensor else None
        in_names, out_names, out_avals, zero_outs = [], [], [], []
        for alloc in nc.m.functions[0].allocations:
            if not isinstance(alloc, mybir.MemoryLocationSet):
                continue
            name = alloc.memorylocations[0].name
            if alloc.kind == "ExternalInput":
                if name != partition_name:
                    in_names.append(name)
            elif alloc.kind == "ExternalOutput":
                shape = tuple(alloc.tensor_shape)
                dtype = mybir.dt.np(alloc.dtype)
                out_names.append(name)
                out_avals.append(jax.core.ShapedArray(shape, dtype))
                zero_outs.append(np.zeros(shape, dtype))
        n_params = len(in_names)
        self.in_names = list(in_names)
        self.out_names = out_names
        self.out_shapes = [tuple(a.shape) for a in out_avals]
        self.zero_outs = zero_outs
        all_in = in_names + out_names + ([partition_name] if partition_name else [])

        def _body(*args):
            operands = list(args)
            if partition_name is not None:
                operands.append(partition_id_tensor())
            return tuple(_bass_exec_p.bind(
                *operands,
                out_avals=tuple(out_avals),
                in_names=tuple(all_in),
                out_names=tuple(out_names),
                lowering_input_output_aliases=(),
                sim_require_finite=True,
                sim_require_nnan=True,
                nc=nc,
            ))

        devices = jax.devices()[:NCORES]
        mesh = Mesh(np.asarray(devices), ("core",))
        in_specs = (PartitionSpec("core"),) * (n_params + len(out_names))
        out_specs = (PartitionSpec("core"),) * len(out_names)
        self.fn = jax.jit(
            shard_map(_body, mesh=mesh, in_specs=in_specs,
                      out_specs=out_specs, check_rep=False),
            keep_unused=True)
        self.jax = jax

    def run(self, in_maps):
        concat_in = [
            np.concatenate([np.asarray(m[nm]) for m in in_maps], axis=0)
            for nm in self.in_names]
        concat_zero = [np.zeros((NCORES * z.shape[0], *z.shape[1:]), z.dtype)
                       for z in self.zero_outs]
        outs = self.fn(*concat_in, *concat_zero)
        return [
            {nm: np.asarray(outs[i]).reshape(NCORES, *self.out_shapes[i])[c]
             for i, nm in enumerate(self.out_names)}
            for c in range(NCORES)]

    def run_timed(self, in_maps, iters=5):
        import time
        concat_in = [
            self.jax.device_put(np.concatenate(
                [np.asarray(m[nm]) for m in in_maps], axis=0))
            for nm in self.in_names]
        concat_zero = [
            self.jax.device_put(
                np.zeros((NCORES * z.shape[0], *z.shape[1:]), z.dtype))
            for z in self.zero_outs]
        o = self.fn(*concat_in, *concat_zero)
        self.jax.block_until_ready(o)
        best = float("inf")
        for _ in range(iters):
            t0 = time.perf_counter()
            o = self.fn(*concat_in, *concat_zero)
            self.jax.block_until_ready(o)
            best = min(best, time.perf_counter() - t0)
        return best


def kernel(**inputs):
    S = inputs["x"].shape[0]
    if S not in _CACHE:
        _CACHE[S] = _Runner(S)
    runner = _CACHE[S]
    outs = runner.run(_in_maps(inputs))
    return _assemble(outs, S)


if __name__ == "__main__":
    rng = np.random.default_rng(0)
    S = 32
    inputs = {"x": rng.standard_normal((S, 64, 512), dtype=np.float32)}
    s = 1.0 / np.sqrt(HID)
    u = lambda *shp: rng.uniform(-s, s, shp).astype(np.float32)
    for c, idim in (("f0", 512), ("b0", 512), ("f1", 1024), ("b1", 1024)):
        inputs[f"wih_{c}"] = u(G3, idim)
        inputs[f"whh_{c}"] = u(G3, HID)
        inputs[f"bih_{c}"] = u(G3)
        inputs[f"bhh_{c}"] = u(G3)
    out = kernel(**inputs)
    print("kernel ran, out", out.shape, float(np.abs(out).mean()))


# revision 32
# speedup vs baseline: 72.1910x; 1.3977x over previous
"""Trainium2 Bass kernel for a 2-layer "BiGRU" (batch-flipped, per reference).

Design (one GRU cell per core, layer-pipelined over 8 cores):
  * The reference's "backward" direction flips the BATCH dim, not time. In
    hat-space (track hb_hat[b] := hb[B-1-b]) every cell consumes the
    UNFLIPPED input stream; the flip appears only in layer-1's input concat
    xi1[j] = [hf0[j], hb_hat0[flip j]] and in the final output assembly.
  * 8 cores = 4 cells x 2 flip-closed batch halves (32 samples each):
      core 0/1: f0 (A/B)   core 2/3: b0 (A/B)
      core 4/5: f1 (A/B)   core 6/7: b1 (A/B)
    Layer-0 cores stream transposed-h chunks to layer-1 cores with a
    per-chunk AllGather over groups {0,2,4,6} / {1,3,5,7}; layer 1 runs one
    chunk behind layer 0 (pipelined via tile deps on the collective).
  * Per-core recurrent work is one cell: 12 whh matmuls + 3 PSUM injects
    per step. Input-side matmuls (x@wihT / xi1@wih1T) are bulk-computed per
    chunk at full PE width from a shared 8-K-chunk lhs buffer (layer 0
    zero-pads chunks 4-7, so the batch-flip view applied to those chunks is
    a no-op) and bounced through DRAM to rotate rows into 32-partition ring
    windows.
  * Gate columns are permuted to [r0 z0 | r1 z1 | n0 n1] so each hidden
    half's gates are contiguous in PSUM and the halves pipeline. Uses
    zm = sigmoid(-z_preact) = 1-z so h' = zm*n + (h - zm*h) with no z
    sigmoid materialized.

Self-contained: hardcodes all shapes from the problem spec.
"""

import contextlib

import numpy as np

from concourse import bacc, tile
from concourse.bass import mybir

SEQ, BATCH, IN, HID = 512, 64, 512, 512
G3 = 3 * HID          # 1536
BC = 32               # batch per core
NCORES = 8
CH = 32               # steps per transfer chunk
W4 = 4                # gi ring window (steps)
FP32 = mybir.dt.float32
BF16 = mybir.dt.bfloat16

# gate-column permutation [r0 z0 | r1 z1 | n0 n1] (256 each)
_PERM = np.concatenate([np.arange(0, 256), np.arange(512, 768),
                        np.arange(256, 512), np.arange(768, 1024),
                        np.arange(1024, 1536)])


def _blob_layout():
    off = {}
    cur = 0
    for k in range(8):
        off[f"w{k}"] = cur; cur += G3          # wihT chunks (L0 uses 0-3)
    for k in range(4):
        off[f"u{k}"] = cur; cur += G3          # whhT chunks
    off["bulkb"] = cur; cur += G3              # row 0: gi bias (permuted)
    off["bhn"] = cur; cur += 512               # row 0: bhh n-part [n0 n1]
    off["ones"] = cur; cur += 128              # row 0: ones
    off["i32"] = cur; cur += 32                # rows 0:32 identity
    off["mcb"] = cur; cur += 128               # (128,128) carry mask (L0=1)
    off["mh"] = cur; cur += 512                # rows 0:32 carry mask (L0=1)
    return off, cur


def build_core_program(S, for_sim=False, role=None):
    """role=None: runtime If on partition id. role=0/1: branchless L0/L1
    variant (for single-core TimelineSim only)."""
    assert S % CH == 0 and CH % W4 == 0 and CH % 8 == 0
    NCHUNK = S // CH
    nc = bacc.Bacc(None, target_bir_lowering=False)
    off, totw = _blob_layout()
    blob_d = nc.declare_dram_parameter("blob", [128, totw // 2], FP32, isOutput=False)
    xTp_d = nc.declare_dram_parameter("xTp", [128, 4, S * BC // 2], FP32, isOutput=False)
    out_d = nc.declare_dram_parameter("out", [S, BC, HID], BF16, isOutput=True)

    ACT = mybir.ActivationFunctionType
    OP = mybir.AluOpType

    with tile.TileContext(nc) as tc:
        ctx = contextlib.ExitStack()
        with ctx:
            const = ctx.enter_context(tc.tile_pool(name="const", bufs=1))
            dram = ctx.enter_context(tc.tile_pool(name="dram", bufs=1, space="DRAM"))
            ghp = ctx.enter_context(tc.tile_pool(name="ghp", bufs=2, space="PSUM"))
            ptrp = ctx.enter_context(tc.tile_pool(name="ptrp", bufs=1, space="PSUM"))
            scr = ctx.enter_context(tc.tile_pool(name="scr", bufs=1, space="PSUM"))
            cb_pool = ctx.enter_context(tc.tile_pool(name="cb", bufs=2))
            ev_pool = ctx.enter_context(tc.tile_pool(name="ev", bufs=2))
            ring_pool = ctx.enter_context(tc.tile_pool(name="ring", bufs=3))
            hs_pool = ctx.enter_context(tc.tile_pool(name="hs", bufs=5))
            g_pool = ctx.enter_context(tc.tile_pool(name="g", bufs=2))

            # ---- constants ----
            blob = const.tile([128, totw // 2], FP32, tag="blob")
            nc.gpsimd.dma_start(out=blob[:], in_=blob_d[:])
            b16 = blob[:].bitcast(BF16)
            W = [b16[:, off[f"w{k}"]:off[f"w{k}"] + G3] for k in range(8)]
            U = [b16[:, off[f"u{k}"]:off[f"u{k}"] + G3] for k in range(4)]
            BULKB = b16[0:1, off["bulkb"]:off["bulkb"] + G3]
            BHN = b16[0:1, off["bhn"]:off["bhn"] + 512]
            ONES = b16[0:1, off["ones"]:off["ones"] + 128]
            IDT = b16[0:32, off["i32"]:off["i32"] + 32]
            MCB = b16[:, off["mcb"]:off["mcb"] + 128]
            MH = b16[0:BC, off["mh"]:off["mh"] + 512]

            zero16 = const.tile([128, 1024], BF16, tag="zero16")
            nc.any.memset(zero16[:], 0.0)

            # persistent double-buffered bulk-lhs: chunks 0-3 (x or hf0T) in
            # lhsbuf, chunks 4-7 (batch-flipped hb0T; zero on layer-0 cores)
            # in flipbuf.
            lhsbuf = [const.tile([128, 4, CH * BC // 2], FP32, tag=f"lhs{i}",
                                 name=f"lhs{i}") for i in range(2)]
            flipbuf = [const.tile([128, 4, CH * BC], BF16, tag=f"flip{i}",
                                  name=f"flip{i}") for i in range(2)]
            for i in range(2):
                nc.any.memset(lhsbuf[i][:], 0.0)
                nc.any.memset(flipbuf[i][:], 0.0)

            # ---- DRAM scratch ----
            gi_dram = [dram.tile([CH * BC, G3], BF16, tag="gi", bufs=4,
                                 name=f"gi{c}") for c in range(NCHUNK + 1)]
            send_dram = [dram.tile([128, 4, CH * BC], BF16, tag="send", bufs=2,
                                   name=f"send{c}") for c in range(NCHUNK)]
            ag_dram = [dram.tile([4, 128, 4, CH * BC], BF16, tag="agout", bufs=2,
                                 name=f"ag{c}") for c in range(NCHUNK)]
            # finite AllGather inputs on non-sender (L1) ranks
            for sl in range(min(2, NCHUNK)):
                for k in range(4):
                    nc.sync.dma_start(out=send_dram[sl][:, k, :],
                                      in_=zero16[:, 0:CH * BC])

            pid = nc.partition_id()

            cbufs = {}
            hstage = {}

            def bulk_chunk(c):
                lhs16 = lhsbuf[c % 2][:].bitcast(BF16)
                fb = flipbuf[c % 2]
                for m in range(CH * BC // 128):
                    ev = ev_pool.tile([128, G3], BF16, tag="ev")
                    for nb in range(3):
                        ps = scr.tile([128, 512], FP32, tag="scr", bufs=1,
                                      name=f"bs{c}_{m}_{nb}")
                        nc.tensor.matmul(out=ps[:], lhsT=ONES,
                                         rhs=BULKB[:, nb * 512:(nb + 1) * 512],
                                         start=True, stop=False)
                        for k in range(8):
                            if k < 4:
                                lhsT = lhs16[:, k, m * 128:(m + 1) * 128]
                            else:
                                lhsT = fb[:, k - 4, m * 128:(m + 1) * 128]
                            nc.tensor.matmul(out=ps[:], lhsT=lhsT,
                                             rhs=W[k][:, nb * 512:(nb + 1) * 512],
                                             start=False, stop=(k == 7))
                        nc.vector.tensor_copy(out=ev[:, nb * 512:(nb + 1) * 512],
                                              in_=ps[:])
                    nc.gpsimd.dma_start(out=gi_dram[c][m * 128:(m + 1) * 128, :],
                                        in_=ev[:])

            def step(t, ring, wi):
                c, s = t // CH, t % CH
                if s == 0:
                    cbufs[c] = cb_pool.tile([128, 4, CH * BC], BF16, tag="cb",
                                            name=f"cb{c}")
                if t % 8 == 0:
                    hstage[t // 8] = hs_pool.tile([BC, 8, HID], BF16, tag="hs",
                                                  name=f"hs{t // 8}")
                hrow = hstage[t // 8][:, t % 8, :]
                hprev = (zero16[0:BC, 0:512] if t == 0
                         else hstage[(t - 1) // 8][:, (t - 1) % 8, :])
                mbb = None
                if t == CH:
                    # layer-1 cores' iteration 0 was a garbage warm-up chunk;
                    # zero their h carry here (mask is 1 on layer-0 cores).
                    mbb = g_pool.tile([128, 4, BC], BF16, tag="mbb", bufs=1)
                    nc.vector.tensor_tensor(
                        out=mbb[:],
                        in0=cbufs[0][:, :, (CH - 1) * BC:CH * BC],
                        in1=MCB.rearrange("p (k b) -> p k b", b=BC), op=OP.mult)
                    hpm = g_pool.tile([BC, 512], BF16, tag="hpm", bufs=1)
                    nc.vector.tensor_tensor(out=hpm[:], in0=hprev, in1=MH,
                                            op=OP.mult)
                    hprev = hpm[:]

                gh = ghp.tile([BC, G3], FP32, tag="gh", name=f"gh{t}")
                rg = ring[:, wi, :]
                for nb in (0, 2, 1):
                    if nb == 2:
                        nc.tensor.matmul(out=gh[:, 1024:1536], lhsT=ONES[:, 0:BC],
                                         rhs=BHN, start=True, stop=False)
                    else:
                        nc.tensor.matmul(out=gh[:, nb * 512:(nb + 1) * 512],
                                         lhsT=IDT,
                                         rhs=rg[:, nb * 512:(nb + 1) * 512],
                                         start=True, stop=False)
                    for k in range(4):
                        if t == 0:
                            lhsT = zero16[:, k * 32:k * 32 + BC]
                        elif t == CH:
                            lhsT = mbb[:, k, :]
                        else:
                            pc, psl = (t - 1) // CH, (t - 1) % CH
                            lhsT = cbufs[pc][:, k, psl * BC:(psl + 1) * BC]
                        nc.tensor.matmul(out=gh[:, nb * 512:(nb + 1) * 512],
                                         lhsT=lhsT,
                                         rhs=U[k][:, nb * 512:(nb + 1) * 512],
                                         start=False, stop=(k == 3))

                pt = ptrp.tile([128, 4, BC], BF16, tag="ptr", bufs=1,
                               name=f"ptr{t}")
                for hf in (0, 1):
                    rb = hf * 512
                    no = 1024 + hf * 256
                    sr = g_pool.tile([BC, 256], BF16, tag=f"sr{hf}")
                    zm = g_pool.tile([BC, 256], BF16, tag=f"zm{hf}")
                    u16 = g_pool.tile([BC, 256], BF16, tag=f"u{hf}")
                    v16 = g_pool.tile([BC, 256], BF16, tag=f"v{hf}")
                    n16 = g_pool.tile([BC, 256], BF16, tag=f"n{hf}")
                    m16 = g_pool.tile([BC, 256], BF16, tag=f"m{hf}")
                    mh = g_pool.tile([BC, 256], BF16, tag=f"mh{hf}")
                    d16 = g_pool.tile([BC, 256], BF16, tag=f"d{hf}")
                    nc.scalar.activation(sr[:], gh[:, rb:rb + 256], ACT.Sigmoid)
                    nc.scalar.activation(zm[:], gh[:, rb + 256:rb + 512],
                                         ACT.Sigmoid, scale=-1.0)
                    nc.vector.tensor_tensor(out=u16[:], in0=sr[:],
                                            in1=gh[:, no:no + 256], op=OP.mult)
                    nc.vector.tensor_tensor(out=v16[:], in0=u16[:],
                                            in1=rg[:, no:no + 256], op=OP.add)
                    nc.scalar.activation(n16[:], v16[:], ACT.Tanh)
                    hp = hprev[:, hf * 256:(hf + 1) * 256]
                    nc.vector.tensor_tensor(out=mh[:], in0=zm[:], in1=hp,
                                            op=OP.mult)
                    nc.vector.tensor_tensor(out=d16[:], in0=hp, in1=mh[:],
                                            op=OP.subtract)
                    nc.vector.tensor_tensor(out=m16[:], in0=zm[:], in1=n16[:],
                                            op=OP.mult)
                    hout = hrow[:, hf * 256:(hf + 1) * 256]
                    nc.vector.tensor_tensor(out=hout, in0=m16[:], in1=d16[:],
                                            op=OP.add)
                    for kk in (0, 1):
                        nc.tensor.transpose(out=pt[:, 2 * hf + kk, :],
                                            in_=hout[:, kk * 128:(kk + 1) * 128],
                                            identity=IDT)
                    nc.vector.tensor_copy(
                        out=cbufs[c][:, 2 * hf:2 * hf + 2, s * BC:(s + 1) * BC],
                        in_=pt[:, 2 * hf:2 * hf + 2, :])

            # ---- chunk pipeline (iteration i: L0 computes chunk i, L1's
            # shared instructions compute its real chunk i-1 from AG_{i-1};
            # L1's iteration 0 is a warm-up on zero inputs, masked off at
            # t==CH; L0's last iteration is a discarded tail). ----
            for i in range(NCHUNK + 1):
                cw = CH * BC // 2
                if i < NCHUNK and role in (None, 0):
                    cm = (tc.If(pid < 4) if role is None
                          else contextlib.nullcontext())
                    with cm:
                        nc.sync.dma_start(out=lhsbuf[i % 2][:, 0:4, :],
                                          in_=xTp_d[:, :, i * cw:(i + 1) * cw])
                if i > 0 and role in (None, 1):
                    cm = (tc.If(pid >= 4) if role is None
                          else contextlib.nullcontext())
                    with cm:
                        v = lhsbuf[i % 2][:].bitcast(BF16)
                        nc.sync.dma_start(out=v[:, 0:4, :], in_=ag_dram[i - 1][0])
                        agt = ev_pool.tile([128, 4, CH * BC], BF16, tag="agt",
                                           name=f"agt{i}")
                        nc.sync.dma_start(out=agt[:], in_=ag_dram[i - 1][1])
                        # batch-flip within each step block (receiver side)
                        nc.vector.tensor_copy(
                            out=flipbuf[i % 2][:].rearrange(
                                "p k (s b) -> p (k s) b", b=BC),
                            in_=agt[:].rearrange(
                                "p k (s b) -> p (k s) b", b=BC)[:, :, ::-1])
                bulk_chunk(i)
                for w in range(CH // W4):
                    r = ring_pool.tile([BC, W4, G3], BF16, tag="ring",
                                       name=f"ring{i}_{w}")
                    src = gi_dram[i][:].rearrange("(s b) g -> b s g", b=BC)
                    nc.sync.dma_start(out=r[:], in_=src[:, w * W4:(w + 1) * W4, :])
                    for j in range(W4):
                        step(i * CH + w * W4 + j, r, j)
                if i < NCHUNK:
                    # L0-only send: L1's AllGather input is the setup-time
                    # zero buffer, so its AG_i has no dependency on its own
                    # iteration-i compute and overlaps it.
                    if role in (None, 0):
                        cm = (tc.If(pid < 4) if role is None
                              else contextlib.nullcontext())
                        with cm:
                            nc.sync.dma_start(out=send_dram[i][:],
                                              in_=cbufs[i][:])
                    nc.gpsimd.collective_compute(
                        "AllGather", mybir.AluOpType.bypass,
                        replica_groups=[[0, 2, 4, 6], [1, 3, 5, 7]],
                        ins=[send_dram[i].opt()], outs=[ag_dram[i].opt()])
                if i > 0 and role in (None, 1):
                    cm = (tc.If(pid >= 4) if role is None
                          else contextlib.nullcontext())
                    with cm:
                        for blk in range(CH // 8):
                            nc.gpsimd.dma_start(
                                out=out_d[:].rearrange(
                                    "(a e) b h -> b a e h", e=8)[:, (i - 1) * (CH // 8) + blk],
                                in_=hstage[i * (CH // 8) + blk][:])
    nc.compile()
    if for_sim:
        nc.insert_bir_kernel_barrier_sem_inc()
    return nc


# ---------------------------------------------------------------------------
# host side
# ---------------------------------------------------------------------------

_CACHE = {}

# batch sets: A = rows 0..15 + 48..63, B = rows 16..47 (both flip-closed,
# local flip = reversal of the 32 local rows)
_SET_A = list(range(16)) + list(range(48, 64))
_SET_B = list(range(16, 48))


def _bf16_u16(a):
    a = np.ascontiguousarray(a, np.float32)
    u = a.view(np.uint32)
    return ((u + 0x7FFF + ((u >> 16) & 1)) >> 16).astype(np.uint16)


def _pack_words(u16):
    ev = u16[..., 0::2].astype(np.uint32)
    od = u16[..., 1::2].astype(np.uint32)
    return (ev | (od << 16)).view(np.float32)


def _blob_host(wih, whh, bih, bhh, carry):
    """Per-core constant blob for one cell. wih: (1536, K) with K in
    {512, 1024}; columns of wihT/whhT permuted to [r0 z0 r1 z1 n0 n1]."""
    off, totw = _blob_layout()
    blob = np.zeros((128, totw), np.uint16)
    K = wih.shape[1]
    wt = wih.T[:, _PERM]                      # (K, 1536) permuted
    kx = K // 128
    wt = _bf16_u16(wt.reshape(kx, 128, G3))
    for k in range(kx):
        blob[:, off[f"w{k}"]:off[f"w{k}"] + G3] = wt[k]
    ut = _bf16_u16(whh.T[:, _PERM].reshape(4, 128, G3))
    for k in range(4):
        blob[:, off[f"u{k}"]:off[f"u{k}"] + G3] = ut[k]
    bulkb = np.concatenate([(bih + bhh)[:1024], bih[1024:]])[_PERM]
    blob[0, off["bulkb"]:off["bulkb"] + G3] = _bf16_u16(bulkb)
    blob[0, off["bhn"]:off["bhn"] + 512] = _bf16_u16(bhh[1024:])
    one = _bf16_u16(np.ones(1, np.float32))[0]
    blob[0, off["ones"]:off["ones"] + 128] = one
    for j in range(32):
        blob[j, off["i32"] + j] = one
    if carry:
        blob[:, off["mcb"]:off["mcb"] + 128] = one
        blob[0:32, off["mh"]:off["mh"] + 512] = one
    return _pack_words(blob)


def _in_maps(inputs):
    S = inputs["x"].shape[0]
    x = np.asarray(inputs["x"], np.float32)
    cells = [("f0", _SET_A), ("f0", _SET_B), ("b0", _SET_A), ("b0", _SET_B),
             ("f1", _SET_A), ("f1", _SET_B), ("b1", _SET_A), ("b1", _SET_B)]
    in_maps = []
    zx = np.zeros((128, 4, S * BC // 2), np.float32)
    for d in range(NCORES):
        cname, bset = cells[d]
        blob = _blob_host(np.asarray(inputs[f"wih_{cname}"], np.float32),
                          np.asarray(inputs[f"whh_{cname}"], np.float32),
                          np.asarray(inputs[f"bih_{cname}"], np.float32),
                          np.asarray(inputs[f"bhh_{cname}"], np.float32),
                          carry=(d < 4))
        if d < 4:
            xl = x[:, bset, :]                               # (S, 32, 512)
            xT = _bf16_u16(xl.transpose(2, 0, 1).reshape(4, 128, S * BC))
            xT = np.ascontiguousarray(xT.transpose(1, 0, 2))  # (128,4,S*BC)
            xw = _pack_words(xT)
        else:
            xw = zx
        in_maps.append({"blob": blob, "xTp": xw})
    return in_maps


def _assemble(outs, S):
    out = np.zeros((S, BATCH, 2 * HID), np.float32)
    for gi_, bset in ((0, _SET_A), (1, _SET_B)):
        f1 = np.asarray(outs[4 + gi_]["out"], np.float32)    # (S, 32, 512)
        b1 = np.asarray(outs[6 + gi_]["out"], np.float32)
        for j, b in enumerate(bset):
            out[:, b, 0:HID] = f1[:, j, :]
            out[:, b, HID:] = b1[:, 31 - j, :]
    return out


class _Runner:
    """Caches the traced+compiled SPMD executable."""

    def __init__(self, S):
        import jax
        from jax.sharding import Mesh, PartitionSpec
        from jax.experimental.shard_map import shard_map
        from concourse import bass2jax
        from concourse.bass2jax import _bass_exec_p, partition_id_tensor

        bass2jax.install_neuronx_cc_hook()
        self.S = S
        nc = build_core_program(S)
        self.nc = nc
        partition_name = nc.partition_id_tensor.name if nc.partition_id_tensor else None
        in_names, out_names, out_avals, zero_outs = [], [], [], []
        for alloc in nc.m.functions[0].allocations:
            if not isinstance(alloc, mybir.MemoryLocationSet):
                continue
            name = alloc.memorylocations[0].name
            if alloc.kind == "ExternalInput":
                if name != partition_name:
                    in_names.append(name)
            elif alloc.kind == "ExternalOutput":
                shape = tuple(alloc.tensor_shape)
                dtype = mybir.dt.np(alloc.dtype)
                out_names.append(name)
                out_avals.append(jax.core.ShapedArray(shape, dtype))
                zero_outs.append(np.zeros(shape, dtype))
        n_params = len(in_names)
        self.in_names = list(in_names)
        self.out_names = out_names
        self.out_shapes = [tuple(a.shape) for a in out_avals]
        self.zero_outs = zero_outs
        all_in = in_names + out_names + ([partition_name] if partition_name else [])

        def _body(*args):
            operands = list(args)
            if partition_name is not None:
                operands.append(partition_id_tensor())
            return tuple(_bass_exec_p.bind(
                *operands,
                out_avals=tuple(out_avals),
                in_names=tuple(all_in),
                out_names=tuple(out_names),
                lowering_input_output_aliases=(),
                sim_require_finite=True,
                sim_require_nnan=True,
                nc=nc,
            ))

        devices = jax.devices()[:NCORES]
        mesh = Mesh(np.asarray(devices), ("core",))
        in_specs = (PartitionSpec("core"),) * (n_params + len(out_names))
        out_specs = (PartitionSpec("core"),) * len(out_names)
        self.fn = jax.jit(
            shard_map(_body, mesh=mesh, in_specs=in_specs,
                      out_specs=out_specs, check_rep=False),
            keep_unused=True)
        self.jax = jax

    def run(self, in_maps):
        concat_in = [
            np.concatenate([np.asarray(m[nm]) for m in in_maps], axis=0)
            for nm in self.in_names]
        concat_zero = [np.zeros((NCORES * z.shape[0], *z.shape[1:]), z.dtype)
                       for z in self.zero_outs]
        outs = self.fn(*concat_in, *concat_zero)
        return [
            {nm: np.asarray(outs[i]).reshape(NCORES, *self.out_shapes[i])[c]
             for i, nm in enumerate(self.out_names)}
            for c in range(NCORES)]

    def run_timed(self, in_maps, iters=5):
        import time
        concat_in = [
            self.jax.device_put(np.concatenate(
                [np.asarray(m[nm]) for m in in_maps], axis=0))
            for nm in self.in_names]
        concat_zero = [
            self.jax.device_put(
                np.zeros((NCORES * z.shape[0], *z.shape[1:]), z.dtype))
            for z in self.zero_outs]
        o = self.fn(*concat_in, *concat_zero)
        self.jax.block_until_ready(o)
        best = float("inf")
        for _ in range(iters):
            t0 = time.perf_counter()
            o = self.fn(*concat_in, *concat_zero)
            self.jax.block_until_ready(o)
            best = min(best, time.perf_counter() - t0)
        return best


def kernel(**inputs):
    S = inputs["x"].shape[0]
    if S not in _CACHE:
        _CACHE[S] = _Runner(S)
    runner = _CACHE[S]
    outs = runner.run(_in_maps(inputs))
    return _assemble(outs, S)


if __name__ == "__main__":
    rng = np.random.default_rng(0)
    S = 32
    inputs = {"x": rng.standard_normal((S, 64, 512), dtype=np.float32)}
    s = 1.0 / np.sqrt(HID)
    u = lambda *shp: rng.uniform(-s, s, shp).astype(np.float32)
    for c, idim in (("f0", 512), ("b0", 512), ("f1", 1024), ("b1", 1024)):
        inputs[f"wih_{c}"] = u(G3, idim)
        inputs[f"whh_{c}"] = u(G3, HID)
        inputs[f"bih_{c}"] = u(G3)
        inputs[f"bhh_{c}"] = u(G3)
    out = kernel(**inputs)
    print("kernel ran, out", out.shape, float(np.abs(out).mean()))
